# revision 9
# baseline (speedup 1.0000x reference)
"""GCN block (2x GCNConv + BatchNorm) on 8 Trainium2 NeuronCores — v3.

Design vs v2:
- The one-hot scatter matrices S (slot-major [slot, tgt], norm baked in) are
  PRECOMPUTED ON HOST and streamed from DRAM per tile via HWDGE DMA instead of
  being built on-device with DVE tensor_scalar ops. v2's trace showed the DVE
  S-builds at ~100% occupancy during both layer phases (the critical path,
  ~1.2 ms of 1.49 ms) while SDMA engines sat ~4% busy. S is pure graph
  structure (one-hot x norm), so host precompute is a format conversion.
- Everything else as v2: per-edge dma_gather from node-major f16 source
  (x16 for layer 1, AllGather'd h1 for layer 2) on 4 SWDGE queues; one-hot
  matmuls accumulate [feat, tgt] in PSUM; one dense matmul per 128-node tile;
  layer 2 feature-major with bias+relu fused in ACT, BN stats via accum_out.
"""

import numpy as np

import concourse.bacc as bacc
import concourse.mybir as mybir
import concourse.tile as tile
from concourse.bass_utils import run_bass_kernel_spmd

N, E, D = 50000, 600000, 128
C = 8                      # cores
NL = N // C                # 6250 nodes per core
T = (NL + 127) // 128      # 49 target tiles per core
LAST = NL - (T - 1) * 128  # 106 valid rows in the last tile
NPAD = ((N + 127) // 128) * 128  # 50048
BUCKET = 32768             # int16-safe source split
EPS = 1e-5

f16 = mybir.dt.float16
f32 = mybir.dt.float32
i16 = mybir.dt.int16

_BUILD_CACHE = {}


# --------------------------------------------------------------------------
# host-side preprocessing (same edge bucketing as v1)
# --------------------------------------------------------------------------

def _prep_edges(edge_index):
    row = np.asarray(edge_index[0], dtype=np.int64)
    col = np.asarray(edge_index[1], dtype=np.int64)
    deg = np.bincount(col, minlength=N).astype(np.float32) + 1.0
    dis = (1.0 / np.sqrt(deg)).astype(np.float32)

    ar = np.arange(N, dtype=np.int64)
    rows = np.concatenate([row, ar])
    cols = np.concatenate([col, ar])
    norm = np.concatenate([dis[row] * dis[col], dis * dis]).astype(np.float32)

    core = cols // NL
    col_loc = cols - core * NL
    t = col_loc >> 7
    b = (rows >= BUCKET).astype(np.int64)
    idxv = (rows - b * BUCKET).astype(np.int16)

    gid = (core * T + t) * 2 + b
    order = np.argsort(gid, kind="stable")
    gid_s = gid[order]
    counts = np.bincount(gid_s, minlength=C * T * 2)
    starts = np.concatenate([[0], np.cumsum(counts)[:-1]])
    rank = (np.arange(len(gid_s)) - starts[gid_s]).astype(np.int64)

    cnt = counts.reshape(C, T, 2)
    NCHA = max(1, int(-(-cnt[:, :, 0].max() // 128)))
    NCHB = max(1, int(-(-cnt[:, :, 1].max() // 128)))
    NCH = NCHA + NCHB
    SW = NCH * 8

    core_s = core[order]
    t_s = t[order]
    b_s = b[order]
    j_s = (col_loc & 127)[order]
    idx_s = idxv[order]
    norm_s = norm[order]

    swt = np.where(b_s == 1, NCHA * 128, 0) + rank
    p = (swt & 127).astype(np.int64)
    ch = (swt >> 7).astype(np.int64)

    # slot-major one-hot scatter matrices with norm baked in:
    # S_host[core, slot, (t*NCH + ch)*128 + tgt_col] = norm
    S_host = np.zeros((C, 128, T * NCH * 128), np.float16)
    S_host[core_s, p, (t_s * NCH + ch) * 128 + j_s] = norm_s

    idxp16 = np.zeros((C, 16, T * SW), np.int16)
    base = np.where(b_s == 1, NCHA * 8, 0)
    idxp16[core_s, rank & 15, t_s * SW + base + (rank >> 4)] = idx_s
    idxp = np.ascontiguousarray(np.tile(idxp16, (1, 8, 1)))

    return NCHA, NCHB, idxp, S_host


# --------------------------------------------------------------------------
# device program
# --------------------------------------------------------------------------

def _build(NCHA, NCHB):
    NCH = NCHA + NCHB
    SW = NCH * 8

    nc = bacc.Bacc("TRN2", target_bir_lowering=False, debug=False,
                   num_devices=C, num_swdge_queues=4)

    def inp(name, shape, dt):
        return nc.dram_tensor(name, shape, dt, kind="ExternalInput").ap()

    x16 = inp("x16", [NPAD, 128], f16)
    W1 = inp("W1", [128, 128], f16)
    W2 = inp("W2", [128, 128], f16)
    b1r = inp("b1r", [1, 128], f16)
    b2c = inp("b2c", [128, 1], f32)
    ones16 = inp("ones16", [1, 128], f16)
    gam = inp("gamma", [128, 1], f32)
    bet = inp("beta", [128, 1], f32)
    idxp = inp("idxp", [128, T * SW], i16)
    Sd = inp("Sd", [128, T * NCH * 128], f16)

    y = nc.dram_tensor("y", [128, NL], f32, kind="ExternalOutput").ap()

    h1_my = nc.dram_tensor("h1_my", [NL, 128], f16)
    h1_all = nc.dram_tensor("h1_all", [N, 128], f16, addr_space="Shared")
    arin = nc.dram_tensor("arin", [128, 2], f32)
    arout = nc.dram_tensor("arout", [128, 2], f32, addr_space="Shared")

    Relu = mybir.ActivationFunctionType.Relu
    Copy = mybir.ActivationFunctionType.Copy
    Ident = mybir.ActivationFunctionType.Identity
    Square = mybir.ActivationFunctionType.Square
    Sqrt = mybir.ActivationFunctionType.Sqrt

    qctr = [0]

    def next_q():
        q = qctr[0] & 3
        qctr[0] += 1
        return q

    with tile.TileContext(nc) as tc:
        with tc.tile_pool(name="const", bufs=1) as cp:
            W1_t = cp.tile([128, 128], f16)
            W2_t = cp.tile([128, 128], f16)
            b1_t = cp.tile([1, 128], f16)
            b2_t = cp.tile([128, 1], f32)
            ones16_t = cp.tile([1, 128], f16)
            gam_t = cp.tile([128, 1], f32)
            bet_t = cp.tile([128, 1], f32)
            idxp_t = cp.tile([128, T * SW], i16)
            h2T = cp.tile([128, T, 128], f16)      # resident layer-2 output
            sums = cp.tile([128, T], f32)          # per-tile feature sums
            sumsq = cp.tile([128, T], f32)         # per-tile feature sum-sq

            for dst, src in [
                (W1_t, W1), (W2_t, W2), (b1_t, b1r), (b2_t, b2c),
                (ones16_t, ones16),
                (gam_t, gam), (bet_t, bet), (idxp_t, idxp),
            ]:
                nc.sync.dma_start(dst[:], src)

            def aggregate_tile(t, srcA, srcB, wp, pp):
                """Gather + one-hot matmuls for target tile t.
                Returns aggT [feat, tgt] f16 SBUF tile."""
                gt = wp.tile([128, NCH, 128], f16, tag="gt")
                nc.gpsimd.dma_gather(
                    gt[:, 0:NCHA, :], srcA,
                    idxp_t[:, t * SW: t * SW + NCHA * 8],
                    NCHA * 128, NCHA * 128, 128, single_packet=False,
                    queue_num=next_q())
                nc.gpsimd.dma_gather(
                    gt[:, NCHA:NCH, :], srcB,
                    idxp_t[:, t * SW + NCHA * 8: (t + 1) * SW],
                    NCHB * 128, NCHB * 128, 128, single_packet=False,
                    queue_num=next_q())
                S = wp.tile([128, NCH, 128], f16, tag="S")
                nc.sync.dma_start(
                    S[:],
                    Sd[:, t * NCH * 128:(t + 1) * NCH * 128].rearrange(
                        "p (c f) -> p c f", c=NCH))
                ps = pp.tile([128, 128], f32, tag="psagg")
                for ch in range(NCH):
                    nc.tensor.matmul(ps[:], gt[:, ch, :], S[:, ch, :],
                                     start=(ch == 0), stop=(ch == NCH - 1))
                aggT = wp.tile([128, 128], f16, tag="aggT")
                nc.scalar.activation(aggT[:], ps[:], Copy)
                return aggT

            # ================= layer 1 =================
            with (
                tc.tile_pool(name="wp1", bufs=6) as wp1,
                tc.tile_pool(name="pp1", bufs=4, space="PSUM") as pp1,
            ):
                stage = [None]
                for t in range(T):
                    aggT = aggregate_tile(t, x16[0:BUCKET, :],
                                          x16[BUCKET:NPAD, :], wp1, pp1)
                    psh = pp1.tile([128, 128], f32, tag="psh")
                    nc.tensor.matmul(psh[:], aggT[:], W1_t[:],
                                     start=True, stop=False)
                    nc.tensor.matmul(psh[:], ones16_t[:], b1_t[:],
                                     start=False, stop=True)
                    i = t % 8
                    if i == 0:
                        stage[0] = wp1.tile([128, 8, 128], f16, tag="h1st",
                                            name="h1st")
                    nc.scalar.activation(stage[0][:, i, :], psh[:], Relu)
                    if i == 7 or t == T - 1:
                        t0 = t - i
                        r0 = t0 * 128
                        nb = i + 1
                        if t < T - 1:
                            dst = h1_my.ap()[r0:r0 + nb * 128, :].rearrange(
                                "(i p) f -> p i f", p=128)
                            nc.sync.dma_start(dst, stage[0][:, 0:nb, :])
                        else:
                            if nb > 1:
                                dst = h1_my.ap()[r0:r0 + (nb - 1) * 128, :]\
                                    .rearrange("(i p) f -> p i f", p=128)
                                nc.sync.dma_start(dst, stage[0][:, 0:nb - 1, :])
                            r1 = r0 + (nb - 1) * 128
                            nc.sync.dma_start(h1_my.ap()[r1:r1 + LAST, :],
                                              stage[0][0:LAST, nb - 1, :])

            nc.gpsimd.collective_compute(
                "AllGather", mybir.AluOpType.bypass,
                replica_groups=[list(range(C))],
                ins=[h1_my.ap()], outs=[h1_all.ap()])

            # ================= layer 2 =================
            with (
                tc.tile_pool(name="wp2", bufs=6) as wp2,
                tc.tile_pool(name="pp2", bufs=4, space="PSUM") as pp2,
            ):
                for t in range(T):
                    aggT = aggregate_tile(t, h1_all.ap()[0:BUCKET, :],
                                          h1_all.ap()[BUCKET:N, :], wp2, pp2)
                    psh = pp2.tile([128, 128], f32, tag="psh2")
                    nc.tensor.matmul(psh[:], W2_t[:], aggT[:],
                                     start=True, stop=True)
                    sqd = wp2.tile([128, 128], f16, tag="sqd")
                    if t < T - 1:
                        nc.scalar.activation(h2T[:, t, :], psh[:], Relu,
                                             bias=b2_t[:],
                                             accum_out=sums[:, t:t + 1])
                        nc.scalar.activation(sqd[:], h2T[:, t, :], Square,
                                             accum_out=sumsq[:, t:t + 1])
                    else:
                        nc.scalar.activation(h2T[:, t, :], psh[:], Relu,
                                             bias=b2_t[:])
                        nc.vector.memset(h2T[:, t, LAST:128], 0.0)
                        nc.scalar.activation(sqd[:], h2T[:, t, :], Ident,
                                             accum_out=sums[:, t:t + 1])
                        nc.scalar.activation(sqd[:], h2T[:, t, :], Square,
                                             accum_out=sumsq[:, t:t + 1])

            # ================= batch norm =================
            with (
                tc.tile_pool(name="wp5", bufs=3) as wp5,
            ):
                acc = wp5.tile([128, 2], f32, tag="acc")
                nc.vector.reduce_sum(acc[:, 0:1], sums[:],
                                     axis=mybir.AxisListType.X)
                nc.vector.reduce_sum(acc[:, 1:2], sumsq[:],
                                     axis=mybir.AxisListType.X)
                nc.sync.dma_start(arin.ap(), acc[:])
                nc.gpsimd.collective_compute(
                    "AllReduce", mybir.AluOpType.add,
                    replica_groups=[list(range(C))],
                    ins=[arin.ap()], outs=[arout.ap()])
                ar = wp5.tile([128, 2], f32, tag="ar")
                nc.sync.dma_start(ar[:], arout.ap())

                mean = wp5.tile([128, 1], f32, tag="mean")
                ex2 = wp5.tile([128, 1], f32, tag="ex2")
                var = wp5.tile([128, 1], f32, tag="var")
                std = wp5.tile([128, 1], f32, tag="std")
                inv = wp5.tile([128, 1], f32, tag="inv")
                scl = wp5.tile([128, 1], f32, tag="scl")
                sft = wp5.tile([128, 1], f32, tag="sft")

                nc.vector.tensor_scalar_mul(mean[:], ar[:, 0:1], 1.0 / N)
                nc.vector.tensor_scalar_mul(ex2[:], ar[:, 1:2], 1.0 / N)
                nc.vector.tensor_mul(var[:], mean[:], mean[:])
                nc.vector.tensor_sub(var[:], ex2[:], var[:])
                nc.vector.tensor_scalar_add(var[:], var[:], EPS)
                nc.scalar.activation(std[:], var[:], Sqrt)
                nc.vector.reciprocal(inv[:], std[:])
                nc.vector.tensor_mul(scl[:], gam_t[:], inv[:])
                nc.vector.tensor_mul(sft[:], mean[:], scl[:])
                nc.vector.tensor_sub(sft[:], bet_t[:], sft[:])

                done = 0
                while done < T:
                    nb = min(8, T - done)
                    yst = wp5.tile([128, 8, 128], f32, tag="yst")
                    for i in range(nb):
                        t = done + i
                        nc.scalar.activation(yst[:, i, :], h2T[:, t, :],
                                             Ident, bias=sft[:], scale=scl[:])
                    c0 = done * 128
                    if done + nb < T:
                        dst = y[:, c0:c0 + nb * 128].rearrange(
                            "p (i f) -> p i f", i=nb)
                        nc.sync.dma_start(dst, yst[:, 0:nb, :])
                    else:
                        if nb > 1:
                            dst = y[:, c0:c0 + (nb - 1) * 128].rearrange(
                                "p (i f) -> p i f", i=nb - 1)
                            nc.sync.dma_start(dst, yst[:, 0:nb - 1, :])
                        c1 = c0 + (nb - 1) * 128
                        nc.sync.dma_start(y[:, c1:c1 + LAST],
                                          yst[:, nb - 1, 0:LAST])
                    done += nb

    nc.compile()
    return nc


# --------------------------------------------------------------------------
# entry point
# --------------------------------------------------------------------------

def _run(inputs, trace=False):
    x = np.asarray(inputs["x"], dtype=np.float32)
    edge_index = np.asarray(inputs["edge_index"])
    W1 = np.asarray(inputs["W1"], dtype=np.float32)
    b1 = np.asarray(inputs["b1"], dtype=np.float32)
    W2 = np.asarray(inputs["W2"], dtype=np.float32)
    b2 = np.asarray(inputs["b2"], dtype=np.float32)
    gamma = np.asarray(inputs["gamma"], dtype=np.float32)
    beta = np.asarray(inputs["beta"], dtype=np.float32)

    NCHA, NCHB, idxp, S_host = _prep_edges(edge_index)
    key = (NCHA, NCHB)
    if key not in _BUILD_CACHE:
        _BUILD_CACHE[key] = _build(NCHA, NCHB)
    nc = _BUILD_CACHE[key]

    xp = np.zeros((NPAD, D), np.float16)
    xp[:N] = x.astype(np.float16)

    common = {
        "x16": xp,
        "W1": W1.astype(np.float16),
        "W2": W2.astype(np.float16),
        "b1r": b1.astype(np.float16)[None, :],
        "b2c": b2.astype(np.float32)[:, None],
        "ones16": np.ones((1, 128), np.float16),
        "gamma": gamma.astype(np.float32).reshape(128, 1),
        "beta": beta.astype(np.float32).reshape(128, 1),
    }
    in_maps = [
        {**common, "idxp": idxp[c], "Sd": S_host[c]}
        for c in range(C)
    ]

    res = run_bass_kernel_spmd(nc, in_maps, list(range(C)), trace=trace)
    out = np.concatenate(
        [np.ascontiguousarray(res.results[c]["y"].T) for c in range(C)], axis=0)
    return out, res


def kernel(**inputs):
    out, _ = _run(inputs, trace=False)
    return out



# revision 17
# speedup vs baseline: 1.0828x; 1.0828x over previous
"""GCN block (2x GCNConv + BatchNorm) on 8 Trainium2 NeuronCores — v4.

Design vs v2:
- S-build batched: the one-hot scatter matrices S [slot, tgt] (norm baked in)
  are built with TWO DVE scalar_tensor_tensor ops per target tile (all NCH
  chunks at once, per-chunk colj/norm broadcast along the free dim via
  stride-0 APs) instead of v2's NCH separate tensor_scalar ops. v2's trace
  showed the per-chunk DVE builds at ~100% occupancy (critical path ~1.2 ms
  of 1.49 ms). Streaming host-built S from DRAM (v3) was tried and is WORSE:
  +50 MB HBM traffic trips the power governor (throttle_active 608 us at 50%
  duty) and skews the AllGather across cores.
- Gather slots sorted by source row within each (tile, bucket) for better
  HBM row locality in the per-edge gathers.
- Everything else as v2: per-edge dma_gather from node-major f16 source
  (x16 for layer 1, AllGather'd h1 for layer 2) on 4 SWDGE queues; one-hot
  matmuls accumulate [feat, tgt] in PSUM; one dense matmul per 128-node tile;
  layer 2 feature-major with bias+relu fused in ACT, BN stats via accum_out.
"""

import numpy as np

import concourse.bacc as bacc
import concourse.mybir as mybir
import concourse.tile as tile
from concourse.bass_utils import run_bass_kernel_spmd

N, E, D = 50000, 600000, 128
C = 8                      # cores
NL = N // C                # 6250 nodes per core
T = (NL + 127) // 128      # 49 target tiles per core
LAST = NL - (T - 1) * 128  # 106 valid rows in the last tile
NPAD = ((N + 127) // 128) * 128  # 50048
BUCKET = 32768             # int16-safe source split
EPS = 1e-5

f16 = mybir.dt.float16
f32 = mybir.dt.float32
i16 = mybir.dt.int16

_BUILD_CACHE = {}


# --------------------------------------------------------------------------
# host-side preprocessing (same edge bucketing as v1)
# --------------------------------------------------------------------------

def _prep_edges(edge_index):
    row = np.asarray(edge_index[0], dtype=np.int64)
    col = np.asarray(edge_index[1], dtype=np.int64)
    deg = np.bincount(col, minlength=N).astype(np.float32) + 1.0
    dis = (1.0 / np.sqrt(deg)).astype(np.float32)

    ar = np.arange(N, dtype=np.int64)
    rows = np.concatenate([row, ar])
    cols = np.concatenate([col, ar])
    norm = np.concatenate([dis[row] * dis[col], dis * dis]).astype(np.float32)

    core = cols // NL
    col_loc = cols - core * NL
    t = col_loc >> 7
    b = (rows >= BUCKET).astype(np.int64)
    idxv = (rows - b * BUCKET).astype(np.int16)

    gid = (core * T + t) * 2 + b
    order = np.lexsort((rows, gid))  # by group, then source row (HBM locality)
    gid_s = gid[order]
    counts = np.bincount(gid_s, minlength=C * T * 2)
    starts = np.concatenate([[0], np.cumsum(counts)[:-1]])
    rank = (np.arange(len(gid_s)) - starts[gid_s]).astype(np.int64)

    cnt = counts.reshape(C, T, 2)
    NCHA = max(1, int(-(-cnt[:, :, 0].max() // 128)))
    NCHB = max(1, int(-(-cnt[:, :, 1].max() // 128)))
    NCH = NCHA + NCHB
    SW = NCH * 8

    core_s = core[order]
    t_s = t[order]
    b_s = b[order]
    j_s = (col_loc & 127)[order]
    idx_s = idxv[order]
    norm_s = norm[order]

    swt = np.where(b_s == 1, NCHA * 128, 0) + rank
    p = (swt & 127).astype(np.int64)
    ch = (swt >> 7).astype(np.int64)

    # per-chunk one-hot params: target column and norm per (slot, tile, chunk)
    colj = np.zeros((C, 128, T * NCH), np.float16)
    normv = np.zeros((C, 128, T * NCH), np.float16)
    colj[core_s, p, t_s * NCH + ch] = j_s
    normv[core_s, p, t_s * NCH + ch] = norm_s.astype(np.float16)

    idxp16 = np.zeros((C, 16, T * SW), np.int16)
    base = np.where(b_s == 1, NCHA * 8, 0)
    idxp16[core_s, rank & 15, t_s * SW + base + (rank >> 4)] = idx_s
    idxp = np.ascontiguousarray(np.tile(idxp16, (1, 8, 1)))

    return NCHA, NCHB, idxp, colj, normv


# --------------------------------------------------------------------------
# device program
# --------------------------------------------------------------------------

def _build(NCHA, NCHB):
    NCH = NCHA + NCHB
    SW = NCH * 8

    nc = bacc.Bacc("TRN2", target_bir_lowering=False, debug=False,
                   num_devices=C, num_swdge_queues=4)

    def inp(name, shape, dt):
        return nc.dram_tensor(name, shape, dt, kind="ExternalInput").ap()

    x16 = inp("x16", [NPAD, 128], f16)
    W1 = inp("W1", [128, 128], f16)
    W2 = inp("W2", [128, 128], f16)
    b1r = inp("b1r", [1, 128], f16)
    b2c = inp("b2c", [128, 1], f32)
    ones16 = inp("ones16", [1, 128], f16)
    gam = inp("gamma", [128, 1], f32)
    bet = inp("beta", [128, 1], f32)
    idxp = inp("idxp", [128, T * SW], i16)
    iotaT = inp("iotaT", [128, NCH * 128], f16)
    colj = inp("colj", [128, T * NCH], f16)
    normv = inp("normv", [128, T * NCH], f16)

    y = nc.dram_tensor("y", [128, NL], f32, kind="ExternalOutput").ap()

    h1_my = nc.dram_tensor("h1_my", [NL, 128], f16)
    h1_all = nc.dram_tensor("h1_all", [N, 128], f16, addr_space="Shared")
    arin = nc.dram_tensor("arin", [128, 2], f32)
    arout = nc.dram_tensor("arout", [128, 2], f32, addr_space="Shared")

    Relu = mybir.ActivationFunctionType.Relu
    Copy = mybir.ActivationFunctionType.Copy
    Ident = mybir.ActivationFunctionType.Identity
    Square = mybir.ActivationFunctionType.Square
    Sqrt = mybir.ActivationFunctionType.Sqrt
    ADD = mybir.AluOpType.add
    EQ = mybir.AluOpType.is_equal
    MUL = mybir.AluOpType.mult

    qctr = [0]

    def next_q():
        q = qctr[0] & 3
        qctr[0] += 1
        return q

    with tile.TileContext(nc) as tc:
        with tc.tile_pool(name="const", bufs=1) as cp:
            W1_t = cp.tile([128, 128], f16)
            W2_t = cp.tile([128, 128], f16)
            b1_t = cp.tile([1, 128], f16)
            b2_t = cp.tile([128, 1], f32)
            ones16_t = cp.tile([1, 128], f16)
            gam_t = cp.tile([128, 1], f32)
            bet_t = cp.tile([128, 1], f32)
            idxp_t = cp.tile([128, T * SW], i16)
            iotaT_t = cp.tile([128, NCH, 128], f16)
            colj_t = cp.tile([128, T * NCH], f16)
            normv_t = cp.tile([128, T * NCH], f16)
            h2T = cp.tile([128, T, 128], f16)      # resident layer-2 output
            sums = cp.tile([128, T], f32)          # per-tile feature sums
            sumsq = cp.tile([128, T], f32)         # per-tile feature sum-sq

            for dst, src in [
                (W1_t, W1), (W2_t, W2), (b1_t, b1r), (b2_t, b2c),
                (ones16_t, ones16),
                (gam_t, gam), (bet_t, bet), (idxp_t, idxp),
                (colj_t, colj), (normv_t, normv),
            ]:
                nc.sync.dma_start(dst[:], src)
            nc.sync.dma_start(
                iotaT_t[:], iotaT.rearrange("p (c f) -> p c f", c=NCH))

            def aggregate_tile(t, srcA, srcB, wp, pp):
                """Gather + one-hot matmuls for target tile t.
                Returns aggT [feat, tgt] f16 SBUF tile."""
                gt = wp.tile([128, NCH, 128], f16, tag="gt")
                nc.gpsimd.dma_gather(
                    gt[:, 0:NCHA, :], srcA,
                    idxp_t[:, t * SW: t * SW + NCHA * 8],
                    NCHA * 128, NCHA * 128, 128, single_packet=False,
                    queue_num=next_q())
                nc.gpsimd.dma_gather(
                    gt[:, NCHA:NCH, :], srcB,
                    idxp_t[:, t * SW + NCHA * 8: (t + 1) * SW],
                    NCHB * 128, NCHB * 128, 128, single_packet=False,
                    queue_num=next_q())
                S = wp.tile([128, NCH, 128], f16, tag="S")
                cjb = colj_t[:, t * NCH:(t + 1) * NCH][:, :, None]\
                    .broadcast_to([128, NCH, 128])
                nvb = normv_t[:, t * NCH:(t + 1) * NCH][:, :, None]\
                    .broadcast_to([128, NCH, 128])
                nc.vector.scalar_tensor_tensor(S[:], iotaT_t[:], 0.0, cjb,
                                               ADD, EQ)
                nc.vector.scalar_tensor_tensor(S[:], S[:], 0.0, nvb,
                                               ADD, MUL)
                ps = pp.tile([128, 128], f32, tag="psagg")
                for ch in range(NCH):
                    nc.tensor.matmul(ps[:], gt[:, ch, :], S[:, ch, :],
                                     start=(ch == 0), stop=(ch == NCH - 1))
                aggT = wp.tile([128, 128], f16, tag="aggT")
                nc.scalar.activation(aggT[:], ps[:], Copy)
                return aggT

            # ================= layer 1 =================
            with (
                tc.tile_pool(name="wp1", bufs=6) as wp1,
                tc.tile_pool(name="pp1", bufs=4, space="PSUM") as pp1,
            ):
                stage = [None]
                for t in range(T):
                    aggT = aggregate_tile(t, x16[0:BUCKET, :],
                                          x16[BUCKET:NPAD, :], wp1, pp1)
                    psh = pp1.tile([128, 128], f32, tag="psh")
                    nc.tensor.matmul(psh[:], aggT[:], W1_t[:],
                                     start=True, stop=False)
                    nc.tensor.matmul(psh[:], ones16_t[:], b1_t[:],
                                     start=False, stop=True)
                    i = t % 8
                    if i == 0:
                        stage[0] = wp1.tile([128, 8, 128], f16, tag="h1st",
                                            name="h1st")
                    nc.scalar.activation(stage[0][:, i, :], psh[:], Relu)
                    if i == 7 or t == T - 1:
                        t0 = t - i
                        r0 = t0 * 128
                        nb = i + 1
                        if t < T - 1:
                            dst = h1_my.ap()[r0:r0 + nb * 128, :].rearrange(
                                "(i p) f -> p i f", p=128)
                            nc.sync.dma_start(dst, stage[0][:, 0:nb, :])
                        else:
                            if nb > 1:
                                dst = h1_my.ap()[r0:r0 + (nb - 1) * 128, :]\
                                    .rearrange("(i p) f -> p i f", p=128)
                                nc.sync.dma_start(dst, stage[0][:, 0:nb - 1, :])
                            r1 = r0 + (nb - 1) * 128
                            nc.sync.dma_start(h1_my.ap()[r1:r1 + LAST, :],
                                              stage[0][0:LAST, nb - 1, :])

            nc.gpsimd.collective_compute(
                "AllGather", mybir.AluOpType.bypass,
                replica_groups=[list(range(C))],
                ins=[h1_my.ap()], outs=[h1_all.ap()])

            # ================= layer 2 =================
            with (
                tc.tile_pool(name="wp2", bufs=6) as wp2,
                tc.tile_pool(name="pp2", bufs=4, space="PSUM") as pp2,
            ):
                for t in range(T):
                    aggT = aggregate_tile(t, h1_all.ap()[0:BUCKET, :],
                                          h1_all.ap()[BUCKET:N, :], wp2, pp2)
                    psh = pp2.tile([128, 128], f32, tag="psh2")
                    nc.tensor.matmul(psh[:], W2_t[:], aggT[:],
                                     start=True, stop=True)
                    sqd = wp2.tile([128, 128], f16, tag="sqd")
                    if t < T - 1:
                        nc.scalar.activation(h2T[:, t, :], psh[:], Relu,
                                             bias=b2_t[:],
                                             accum_out=sums[:, t:t + 1])
                        nc.scalar.activation(sqd[:], h2T[:, t, :], Square,
                                             accum_out=sumsq[:, t:t + 1])
                    else:
                        nc.scalar.activation(h2T[:, t, :], psh[:], Relu,
                                             bias=b2_t[:])
                        nc.vector.memset(h2T[:, t, LAST:128], 0.0)
                        nc.scalar.activation(sqd[:], h2T[:, t, :], Ident,
                                             accum_out=sums[:, t:t + 1])
                        nc.scalar.activation(sqd[:], h2T[:, t, :], Square,
                                             accum_out=sumsq[:, t:t + 1])

            # ================= batch norm =================
            with (
                tc.tile_pool(name="wp5", bufs=3) as wp5,
            ):
                acc = wp5.tile([128, 2], f32, tag="acc")
                nc.vector.reduce_sum(acc[:, 0:1], sums[:],
                                     axis=mybir.AxisListType.X)
                nc.vector.reduce_sum(acc[:, 1:2], sumsq[:],
                                     axis=mybir.AxisListType.X)
                nc.sync.dma_start(arin.ap(), acc[:])
                nc.gpsimd.collective_compute(
                    "AllReduce", mybir.AluOpType.add,
                    replica_groups=[list(range(C))],
                    ins=[arin.ap()], outs=[arout.ap()])
                ar = wp5.tile([128, 2], f32, tag="ar")
                nc.sync.dma_start(ar[:], arout.ap())

                mean = wp5.tile([128, 1], f32, tag="mean")
                ex2 = wp5.tile([128, 1], f32, tag="ex2")
                var = wp5.tile([128, 1], f32, tag="var")
                std = wp5.tile([128, 1], f32, tag="std")
                inv = wp5.tile([128, 1], f32, tag="inv")
                scl = wp5.tile([128, 1], f32, tag="scl")
                sft = wp5.tile([128, 1], f32, tag="sft")

                nc.vector.tensor_scalar_mul(mean[:], ar[:, 0:1], 1.0 / N)
                nc.vector.tensor_scalar_mul(ex2[:], ar[:, 1:2], 1.0 / N)
                nc.vector.tensor_mul(var[:], mean[:], mean[:])
                nc.vector.tensor_sub(var[:], ex2[:], var[:])
                nc.vector.tensor_scalar_add(var[:], var[:], EPS)
                nc.scalar.activation(std[:], var[:], Sqrt)
                nc.vector.reciprocal(inv[:], std[:])
                nc.vector.tensor_mul(scl[:], gam_t[:], inv[:])
                nc.vector.tensor_mul(sft[:], mean[:], scl[:])
                nc.vector.tensor_sub(sft[:], bet_t[:], sft[:])

                done = 0
                while done < T:
                    nb = min(8, T - done)
                    yst = wp5.tile([128, 8, 128], f32, tag="yst")
                    for i in range(nb):
                        t = done + i
                        nc.scalar.activation(yst[:, i, :], h2T[:, t, :],
                                             Ident, bias=sft[:], scale=scl[:])
                    c0 = done * 128
                    if done + nb < T:
                        dst = y[:, c0:c0 + nb * 128].rearrange(
                            "p (i f) -> p i f", i=nb)
                        nc.sync.dma_start(dst, yst[:, 0:nb, :])
                    else:
                        if nb > 1:
                            dst = y[:, c0:c0 + (nb - 1) * 128].rearrange(
                                "p (i f) -> p i f", i=nb - 1)
                            nc.sync.dma_start(dst, yst[:, 0:nb - 1, :])
                        c1 = c0 + (nb - 1) * 128
                        nc.sync.dma_start(y[:, c1:c1 + LAST],
                                          yst[:, nb - 1, 0:LAST])
                    done += nb

    nc.compile()
    return nc


# --------------------------------------------------------------------------
# entry point
# --------------------------------------------------------------------------

def _run(inputs, trace=False):
    x = np.asarray(inputs["x"], dtype=np.float32)
    edge_index = np.asarray(inputs["edge_index"])
    W1 = np.asarray(inputs["W1"], dtype=np.float32)
    b1 = np.asarray(inputs["b1"], dtype=np.float32)
    W2 = np.asarray(inputs["W2"], dtype=np.float32)
    b2 = np.asarray(inputs["b2"], dtype=np.float32)
    gamma = np.asarray(inputs["gamma"], dtype=np.float32)
    beta = np.asarray(inputs["beta"], dtype=np.float32)

    NCHA, NCHB, idxp, colj, normv = _prep_edges(edge_index)
    key = (NCHA, NCHB)
    if key not in _BUILD_CACHE:
        _BUILD_CACHE[key] = _build(NCHA, NCHB)
    nc = _BUILD_CACHE[key]

    xp = np.zeros((NPAD, D), np.float16)
    xp[:N] = x.astype(np.float16)

    NCH = NCHA + NCHB
    iotaT = np.ascontiguousarray(
        np.tile(np.arange(128, dtype=np.float16), (128, NCH)))

    common = {
        "x16": xp,
        "W1": W1.astype(np.float16),
        "W2": W2.astype(np.float16),
        "b1r": b1.astype(np.float16)[None, :],
        "b2c": b2.astype(np.float32)[:, None],
        "ones16": np.ones((1, 128), np.float16),
        "gamma": gamma.astype(np.float32).reshape(128, 1),
        "beta": beta.astype(np.float32).reshape(128, 1),
        "iotaT": iotaT,
    }
    in_maps = [
        {**common, "idxp": idxp[c], "colj": colj[c], "normv": normv[c]}
        for c in range(C)
    ]

    res = run_bass_kernel_spmd(nc, in_maps, list(range(C)), trace=trace)
    out = np.concatenate(
        [np.ascontiguousarray(res.results[c]["y"].T) for c in range(C)], axis=0)
    return out, res


def kernel(**inputs):
    out, _ = _run(inputs, trace=False)
    return out



# revision 20
# speedup vs baseline: 1.3590x; 1.2551x over previous
"""GCN block (2x GCNConv + BatchNorm) on 8 Trainium2 NeuronCores — v4.

Design vs v2:
- S-build batched: the one-hot scatter matrices S [slot, tgt] (norm baked in)
  are built with TWO DVE scalar_tensor_tensor ops per target tile (all NCH
  chunks at once, per-chunk colj/norm broadcast along the free dim via
  stride-0 APs) instead of v2's NCH separate tensor_scalar ops. v2's trace
  showed the per-chunk DVE builds at ~100% occupancy (critical path ~1.2 ms
  of 1.49 ms). Streaming host-built S from DRAM (v3) was tried and is WORSE:
  +50 MB HBM traffic trips the power governor (throttle_active 608 us at 50%
  duty) and skews the AllGather across cores.
- Gather slots sorted by source row within each (tile, bucket) for better
  HBM row locality in the per-edge gathers.
- Everything else as v2: per-edge dma_gather from node-major f16 source
  (x16 for layer 1, AllGather'd h1 for layer 2) on 4 SWDGE queues; one-hot
  matmuls accumulate [feat, tgt] in PSUM; one dense matmul per 128-node tile;
  layer 2 feature-major with bias+relu fused in ACT, BN stats via accum_out.
"""

import numpy as np

import concourse.bacc as bacc
import concourse.mybir as mybir
import concourse.tile as tile
from concourse.bass_utils import run_bass_kernel_spmd

N, E, D = 50000, 600000, 128
C = 8                      # cores
NL = N // C                # 6250 nodes per core
T = (NL + 127) // 128      # 49 target tiles per core
LAST = NL - (T - 1) * 128  # 106 valid rows in the last tile
NPAD = ((N + 127) // 128) * 128  # 50048
BUCKET = 32768             # int16-safe source split
EPS = 1e-5

f16 = mybir.dt.float16
f32 = mybir.dt.float32
i16 = mybir.dt.int16

_BUILD_CACHE = {}


# --------------------------------------------------------------------------
# host-side preprocessing (same edge bucketing as v1)
# --------------------------------------------------------------------------

def _prep_edges(edge_index):
    row = np.asarray(edge_index[0], dtype=np.int64)
    col = np.asarray(edge_index[1], dtype=np.int64)
    deg = np.bincount(col, minlength=N).astype(np.float32) + 1.0
    dis = (1.0 / np.sqrt(deg)).astype(np.float32)

    ar = np.arange(N, dtype=np.int64)
    rows = np.concatenate([row, ar])
    cols = np.concatenate([col, ar])
    norm = np.concatenate([dis[row] * dis[col], dis * dis]).astype(np.float32)

    core = cols // NL
    col_loc = cols - core * NL
    t = col_loc >> 7
    b = (rows >= BUCKET).astype(np.int64)
    idxv = (rows - b * BUCKET).astype(np.int16)

    gid = (core * T + t) * 2 + b
    order = np.lexsort((rows, gid))  # by group, then source row (HBM locality)
    gid_s = gid[order]
    counts = np.bincount(gid_s, minlength=C * T * 2)
    starts = np.concatenate([[0], np.cumsum(counts)[:-1]])
    rank = (np.arange(len(gid_s)) - starts[gid_s]).astype(np.int64)

    cnt = counts.reshape(C, T, 2)
    NCHA = max(1, int(-(-cnt[:, :, 0].max() // 128)))
    NCHB = max(1, int(-(-cnt[:, :, 1].max() // 128)))
    NCH = NCHA + NCHB
    SW = NCH * 8

    core_s = core[order]
    t_s = t[order]
    b_s = b[order]
    j_s = (col_loc & 127)[order]
    idx_s = idxv[order]
    norm_s = norm[order]

    swt = np.where(b_s == 1, NCHA * 128, 0) + rank
    p = (swt & 127).astype(np.int64)
    ch = (swt >> 7).astype(np.int64)

    # per-chunk one-hot params: target column and norm per (slot, tile, chunk)
    colj = np.zeros((C, 128, T * NCH), np.float16)
    normv = np.zeros((C, 128, T * NCH), np.float16)
    colj[core_s, p, t_s * NCH + ch] = j_s
    normv[core_s, p, t_s * NCH + ch] = norm_s.astype(np.float16)

    idxp16 = np.zeros((C, 16, T * SW), np.int16)
    base = np.where(b_s == 1, NCHA * 8, 0)
    idxp16[core_s, rank & 15, t_s * SW + base + (rank >> 4)] = idx_s
    idxp = np.ascontiguousarray(np.tile(idxp16, (1, 8, 1)))

    return NCHA, NCHB, idxp, colj, normv


# --------------------------------------------------------------------------
# device program
# --------------------------------------------------------------------------

def _build(NCHA, NCHB):
    NCH = NCHA + NCHB
    SW = NCH * 8

    nc = bacc.Bacc("TRN2", target_bir_lowering=False, debug=False,
                   num_devices=C, num_swdge_queues=4,
                   dynamic_dma_scratch_size=49152)

    def inp(name, shape, dt):
        return nc.dram_tensor(name, shape, dt, kind="ExternalInput").ap()

    x16 = inp("x16", [NPAD, 128], f16)
    W1 = inp("W1", [128, 128], f16)
    W2 = inp("W2", [128, 128], f16)
    b1r = inp("b1r", [1, 128], f16)
    b2c = inp("b2c", [128, 1], f32)
    ones16 = inp("ones16", [1, 128], f16)
    gam = inp("gamma", [128, 1], f32)
    bet = inp("beta", [128, 1], f32)
    idxp = inp("idxp", [128, T * SW], i16)
    iotaT = inp("iotaT", [128, NCH * 128], f16)
    colj = inp("colj", [128, T * NCH], f16)
    normv = inp("normv", [128, T * NCH], f16)

    y = nc.dram_tensor("y", [128, NL], f32, kind="ExternalOutput").ap()

    h1_my = nc.dram_tensor("h1_my", [NL, 128], f16)
    h1_all = nc.dram_tensor("h1_all", [N, 128], f16, addr_space="Shared")
    arin = nc.dram_tensor("arin", [128, 2], f32)
    arout = nc.dram_tensor("arout", [128, 2], f32, addr_space="Shared")

    Relu = mybir.ActivationFunctionType.Relu
    Copy = mybir.ActivationFunctionType.Copy
    Ident = mybir.ActivationFunctionType.Identity
    Square = mybir.ActivationFunctionType.Square
    Sqrt = mybir.ActivationFunctionType.Sqrt
    ADD = mybir.AluOpType.add
    EQ = mybir.AluOpType.is_equal
    MUL = mybir.AluOpType.mult

    qctr = [0]

    def next_q():
        q = qctr[0] & 3
        qctr[0] += 1
        return q

    with tile.TileContext(nc) as tc:
        with tc.tile_pool(name="const", bufs=1) as cp:
            W1_t = cp.tile([128, 128], f16)
            W2_t = cp.tile([128, 128], f16)
            b1_t = cp.tile([1, 128], f16)
            b2_t = cp.tile([128, 1], f32)
            ones16_t = cp.tile([1, 128], f16)
            gam_t = cp.tile([128, 1], f32)
            bet_t = cp.tile([128, 1], f32)
            idxp_t = cp.tile([128, T * SW], i16)
            iotaT_t = cp.tile([128, NCH, 128], f16)
            colj_t = cp.tile([128, T * NCH], f16)
            normv_t = cp.tile([128, T * NCH], f16)
            h2T = cp.tile([128, T, 128], f16)      # resident layer-2 output
            sums = cp.tile([128, T], f32)          # per-tile feature sums
            sumsq = cp.tile([128, T], f32)         # per-tile feature sum-sq

            for dst, src in [
                (W1_t, W1), (W2_t, W2), (b1_t, b1r), (b2_t, b2c),
                (ones16_t, ones16),
                (gam_t, gam), (bet_t, bet), (idxp_t, idxp),
                (colj_t, colj), (normv_t, normv),
            ]:
                nc.sync.dma_start(dst[:], src)
            nc.sync.dma_start(
                iotaT_t[:], iotaT.rearrange("p (c f) -> p c f", c=NCH))

            def aggregate_tile(t, srcA, srcB, wp, pp):
                """Gather + one-hot matmuls for target tile t.
                Returns aggT [feat, tgt] f16 SBUF tile."""
                gt = wp.tile([128, NCH, 128], f16, tag="gt")
                nc.gpsimd.dma_gather(
                    gt[:, 0:NCHA, :], srcA,
                    idxp_t[:, t * SW: t * SW + NCHA * 8],
                    NCHA * 128, NCHA * 128, 128, single_packet=False,
                    queue_num=next_q())
                nc.gpsimd.dma_gather(
                    gt[:, NCHA:NCH, :], srcB,
                    idxp_t[:, t * SW + NCHA * 8: (t + 1) * SW],
                    NCHB * 128, NCHB * 128, 128, single_packet=False,
                    queue_num=next_q())
                S = wp.tile([128, NCH, 128], f16, tag="S")
                cjb = colj_t[:, t * NCH:(t + 1) * NCH][:, :, None]\
                    .broadcast_to([128, NCH, 128])
                nvb = normv_t[:, t * NCH:(t + 1) * NCH][:, :, None]\
                    .broadcast_to([128, NCH, 128])
                nc.vector.scalar_tensor_tensor(S[:], iotaT_t[:], 0.0, cjb,
                                               ADD, EQ)
                nc.vector.scalar_tensor_tensor(S[:], S[:], 0.0, nvb,
                                               ADD, MUL)
                ps = pp.tile([128, 128], f32, tag="psagg")
                for ch in range(NCH):
                    nc.tensor.matmul(ps[:], gt[:, ch, :], S[:, ch, :],
                                     start=(ch == 0), stop=(ch == NCH - 1))
                aggT = wp.tile([128, 128], f16, tag="aggT")
                nc.scalar.activation(aggT[:], ps[:], Copy)
                return aggT

            # ================= layer 1 =================
            with (
                tc.tile_pool(name="wp1", bufs=6) as wp1,
                tc.tile_pool(name="pp1", bufs=4, space="PSUM") as pp1,
            ):
                stage = [None]
                for t in range(T):
                    aggT = aggregate_tile(t, x16[0:BUCKET, :],
                                          x16[BUCKET:NPAD, :], wp1, pp1)
                    psh = pp1.tile([128, 128], f32, tag="psh")
                    nc.tensor.matmul(psh[:], aggT[:], W1_t[:],
                                     start=True, stop=False)
                    nc.tensor.matmul(psh[:], ones16_t[:], b1_t[:],
                                     start=False, stop=True)
                    i = t % 8
                    if i == 0:
                        stage[0] = wp1.tile([128, 8, 128], f16, tag="h1st",
                                            name="h1st")
                    nc.scalar.activation(stage[0][:, i, :], psh[:], Relu)
                    if i == 7 or t == T - 1:
                        t0 = t - i
                        r0 = t0 * 128
                        nb = i + 1
                        if t < T - 1:
                            dst = h1_my.ap()[r0:r0 + nb * 128, :].rearrange(
                                "(i p) f -> p i f", p=128)
                            nc.sync.dma_start(dst, stage[0][:, 0:nb, :])
                        else:
                            if nb > 1:
                                dst = h1_my.ap()[r0:r0 + (nb - 1) * 128, :]\
                                    .rearrange("(i p) f -> p i f", p=128)
                                nc.sync.dma_start(dst, stage[0][:, 0:nb - 1, :])
                            r1 = r0 + (nb - 1) * 128
                            nc.sync.dma_start(h1_my.ap()[r1:r1 + LAST, :],
                                              stage[0][0:LAST, nb - 1, :])

            nc.gpsimd.collective_compute(
                "AllGather", mybir.AluOpType.bypass,
                replica_groups=[list(range(C))],
                ins=[h1_my.ap()], outs=[h1_all.ap()])

            # ================= layer 2 =================
            with (
                tc.tile_pool(name="wp2", bufs=6) as wp2,
                tc.tile_pool(name="pp2", bufs=4, space="PSUM") as pp2,
            ):
                for t in range(T):
                    aggT = aggregate_tile(t, h1_all.ap()[0:BUCKET, :],
                                          h1_all.ap()[BUCKET:N, :], wp2, pp2)
                    psh = pp2.tile([128, 128], f32, tag="psh2")
                    nc.tensor.matmul(psh[:], W2_t[:], aggT[:],
                                     start=True, stop=True)
                    sqd = wp2.tile([128, 128], f16, tag="sqd")
                    if t < T - 1:
                        nc.scalar.activation(h2T[:, t, :], psh[:], Relu,
                                             bias=b2_t[:],
                                             accum_out=sums[:, t:t + 1])
                        nc.scalar.activation(sqd[:], h2T[:, t, :], Square,
                                             accum_out=sumsq[:, t:t + 1])
                    else:
                        nc.scalar.activation(h2T[:, t, :], psh[:], Relu,
                                             bias=b2_t[:])
                        nc.vector.memset(h2T[:, t, LAST:128], 0.0)
                        nc.scalar.activation(sqd[:], h2T[:, t, :], Ident,
                                             accum_out=sums[:, t:t + 1])
                        nc.scalar.activation(sqd[:], h2T[:, t, :], Square,
                                             accum_out=sumsq[:, t:t + 1])

            # ================= batch norm =================
            with (
                tc.tile_pool(name="wp5", bufs=3) as wp5,
            ):
                acc = wp5.tile([128, 2], f32, tag="acc")
                nc.vector.reduce_sum(acc[:, 0:1], sums[:],
                                     axis=mybir.AxisListType.X)
                nc.vector.reduce_sum(acc[:, 1:2], sumsq[:],
                                     axis=mybir.AxisListType.X)
                nc.sync.dma_start(arin.ap(), acc[:])
                nc.gpsimd.collective_compute(
                    "AllReduce", mybir.AluOpType.add,
                    replica_groups=[list(range(C))],
                    ins=[arin.ap()], outs=[arout.ap()])
                ar = wp5.tile([128, 2], f32, tag="ar")
                nc.sync.dma_start(ar[:], arout.ap())

                mean = wp5.tile([128, 1], f32, tag="mean")
                ex2 = wp5.tile([128, 1], f32, tag="ex2")
                var = wp5.tile([128, 1], f32, tag="var")
                std = wp5.tile([128, 1], f32, tag="std")
                inv = wp5.tile([128, 1], f32, tag="inv")
                scl = wp5.tile([128, 1], f32, tag="scl")
                sft = wp5.tile([128, 1], f32, tag="sft")

                nc.vector.tensor_scalar_mul(mean[:], ar[:, 0:1], 1.0 / N)
                nc.vector.tensor_scalar_mul(ex2[:], ar[:, 1:2], 1.0 / N)
                nc.vector.tensor_mul(var[:], mean[:], mean[:])
                nc.vector.tensor_sub(var[:], ex2[:], var[:])
                nc.vector.tensor_scalar_add(var[:], var[:], EPS)
                nc.scalar.activation(std[:], var[:], Sqrt)
                nc.vector.reciprocal(inv[:], std[:])
                nc.vector.tensor_mul(scl[:], gam_t[:], inv[:])
                nc.vector.tensor_mul(sft[:], mean[:], scl[:])
                nc.vector.tensor_sub(sft[:], bet_t[:], sft[:])

                done = 0
                while done < T:
                    nb = min(8, T - done)
                    yst = wp5.tile([128, 8, 128], f32, tag="yst")
                    for i in range(nb):
                        t = done + i
                        nc.scalar.activation(yst[:, i, :], h2T[:, t, :],
                                             Ident, bias=sft[:], scale=scl[:])
                    c0 = done * 128
                    if done + nb < T:
                        dst = y[:, c0:c0 + nb * 128].rearrange(
                            "p (i f) -> p i f", i=nb)
                        nc.sync.dma_start(dst, yst[:, 0:nb, :])
                    else:
                        if nb > 1:
                            dst = y[:, c0:c0 + (nb - 1) * 128].rearrange(
                                "p (i f) -> p i f", i=nb - 1)
                            nc.sync.dma_start(dst, yst[:, 0:nb - 1, :])
                        c1 = c0 + (nb - 1) * 128
                        nc.sync.dma_start(y[:, c1:c1 + LAST],
                                          yst[:, nb - 1, 0:LAST])
                    done += nb

    nc.compile()
    return nc


# --------------------------------------------------------------------------
# entry point
# --------------------------------------------------------------------------

def _run(inputs, trace=False):
    x = np.asarray(inputs["x"], dtype=np.float32)
    edge_index = np.asarray(inputs["edge_index"])
    W1 = np.asarray(inputs["W1"], dtype=np.float32)
    b1 = np.asarray(inputs["b1"], dtype=np.float32)
    W2 = np.asarray(inputs["W2"], dtype=np.float32)
    b2 = np.asarray(inputs["b2"], dtype=np.float32)
    gamma = np.asarray(inputs["gamma"], dtype=np.float32)
    beta = np.asarray(inputs["beta"], dtype=np.float32)

    NCHA, NCHB, idxp, colj, normv = _prep_edges(edge_index)
    key = (NCHA, NCHB)
    if key not in _BUILD_CACHE:
        _BUILD_CACHE[key] = _build(NCHA, NCHB)
    nc = _BUILD_CACHE[key]

    xp = np.zeros((NPAD, D), np.float16)
    xp[:N] = x.astype(np.float16)

    NCH = NCHA + NCHB
    iotaT = np.ascontiguousarray(
        np.tile(np.arange(128, dtype=np.float16), (128, NCH)))

    common = {
        "x16": xp,
        "W1": W1.astype(np.float16),
        "W2": W2.astype(np.float16),
        "b1r": b1.astype(np.float16)[None, :],
        "b2c": b2.astype(np.float32)[:, None],
        "ones16": np.ones((1, 128), np.float16),
        "gamma": gamma.astype(np.float32).reshape(128, 1),
        "beta": beta.astype(np.float32).reshape(128, 1),
        "iotaT": iotaT,
    }
    in_maps = [
        {**common, "idxp": idxp[c], "colj": colj[c], "normv": normv[c]}
        for c in range(C)
    ]

    res = run_bass_kernel_spmd(nc, in_maps, list(range(C)), trace=trace)
    out = np.concatenate(
        [np.ascontiguousarray(res.results[c]["y"].T) for c in range(C)], axis=0)
    return out, res


def kernel(**inputs):
    out, _ = _run(inputs, trace=False)
    return out



# revision 32
# speedup vs baseline: 1.7355x; 1.2770x over previous
"""GCN block (2x GCNConv + BatchNorm) on 8 Trainium2 NeuronCores — v4.

Design vs v2:
- S-build batched: the one-hot scatter matrices S [slot, tgt] (norm baked in)
  are built with TWO DVE scalar_tensor_tensor ops per target tile (all NCH
  chunks at once, per-chunk colj/norm broadcast along the free dim via
  stride-0 APs) instead of v2's NCH separate tensor_scalar ops. v2's trace
  showed the per-chunk DVE builds at ~100% occupancy (critical path ~1.2 ms
  of 1.49 ms). Streaming host-built S from DRAM (v3) was tried and is WORSE:
  +50 MB HBM traffic trips the power governor (throttle_active 608 us at 50%
  duty) and skews the AllGather across cores.
- Gather slots sorted by source row within each (tile, bucket) for better
  HBM row locality in the per-edge gathers.
- Everything else as v2: per-edge dma_gather from node-major f16 source
  (x16 for layer 1, AllGather'd h1 for layer 2) on 4 SWDGE queues; one-hot
  matmuls accumulate [feat, tgt] in PSUM; one dense matmul per 128-node tile;
  layer 2 feature-major with bias+relu fused in ACT, BN stats via accum_out.
"""

import numpy as np

import concourse.bacc as bacc
import concourse.mybir as mybir
import concourse.tile as tile
from concourse.bass_utils import run_bass_kernel_spmd

N, E, D = 50000, 600000, 128
C = 8                      # cores
NL = N // C                # 6250 nodes per core
T = (NL + 127) // 128      # 49 target tiles per core
LAST = NL - (T - 1) * 128  # 106 valid rows in the last tile
NPAD = ((N + 127) // 128) * 128  # 50048
BUCKET = 32768             # int16-safe source split
EPS = 1e-5

f16 = mybir.dt.float16
f32 = mybir.dt.float32
i16 = mybir.dt.int16

_BUILD_CACHE = {}


# --------------------------------------------------------------------------
# host-side preprocessing (same edge bucketing as v1)
# --------------------------------------------------------------------------

def _prep_edges(edge_index):
    row = np.asarray(edge_index[0], dtype=np.int64)
    col = np.asarray(edge_index[1], dtype=np.int64)
    deg = np.bincount(col, minlength=N).astype(np.float32) + 1.0
    dis = (1.0 / np.sqrt(deg)).astype(np.float32)

    # real edges only; self-loops handled by a dense diagonal chunk
    rows = row
    cols = col
    norm = (dis[row] * dis[col]).astype(np.float32)

    core = cols // NL
    col_loc = cols - core * NL
    t = col_loc >> 7
    b = (rows >= BUCKET).astype(np.int64)
    idxv = (rows - b * BUCKET).astype(np.int16)

    gid = (core * T + t) * 2 + b
    order = np.lexsort((rows, gid))  # by group, then source row
    gid_s = gid[order]
    counts = np.bincount(gid_s, minlength=C * T * 2)
    starts = np.concatenate([[0], np.cumsum(counts)[:-1]])
    rank = (np.arange(len(gid_s)) - starts[gid_s]).astype(np.int64)

    cnt = counts.reshape(C, T, 2)
    NCHA = max(1, int(-(-cnt[:, :, 0].max() // 128)))
    NCHB = max(1, int(-(-cnt[:, :, 1].max() // 128)))
    NCH = NCHA + NCHB + 1      # +1: dense self-loop (diagonal) chunk
    SW = (NCHA + NCHB) * 8

    core_s = core[order]
    t_s = t[order]
    b_s = b[order]
    j_s = (col_loc & 127)[order]
    idx_s = idxv[order]
    norm_s = norm[order]

    swt = np.where(b_s == 1, NCHA * 128, 0) + rank
    p = (swt & 127).astype(np.int64)
    ch = (swt >> 7).astype(np.int64)

    # per-chunk one-hot params: target column and norm per (slot, tile, chunk)
    colj = np.zeros((C, 128, T * NCH), np.float16)
    normv = np.zeros((C, 128, T * NCH), np.float16)
    colj[core_s, p, t_s * NCH + ch] = j_s
    normv[core_s, p, t_s * NCH + ch] = norm_s.astype(np.float16)

    # diagonal self-loop chunk: slot p aggregates into target column p with
    # weight dis^2 of the tile's own node (0 on pad rows of the last tile)
    pp = np.arange(128)
    colj[:, :, NCH - 1::NCH] = pp[None, :, None].astype(np.float16)
    dis2 = np.zeros((C, T * 128), np.float32)
    dis2[:, :NL] = (dis * dis).reshape(C, NL)
    normv[:, :, NCH - 1::NCH] = dis2.reshape(C, T, 128).transpose(
        0, 2, 1).astype(np.float16)

    idxp16 = np.zeros((C, 16, T * SW), np.int16)
    base = np.where(b_s == 1, NCHA * 8, 0)
    idxp16[core_s, rank & 15, t_s * SW + base + (rank >> 4)] = idx_s
    idxp = np.ascontiguousarray(np.tile(idxp16, (1, 8, 1)))

    # exact per-(tile, bucket) gather counts (>=1 so the ucode path is sane)
    cnts = np.maximum(cnt, 1).astype(np.int32)  # [C, T, 2]

    return NCHA, NCHB, idxp, colj, normv, cnts


# --------------------------------------------------------------------------
# device program
# --------------------------------------------------------------------------

def _build(NCHA, NCHB):
    NCH = NCHA + NCHB + 1      # +1: dense self-loop (diagonal) chunk
    SW = (NCHA + NCHB) * 8

    nc = bacc.Bacc("TRN2", target_bir_lowering=False, debug=False,
                   num_devices=C, num_swdge_queues=4,
                   dynamic_dma_scratch_size=49152)

    def inp(name, shape, dt):
        return nc.dram_tensor(name, shape, dt, kind="ExternalInput").ap()

    x16 = inp("x16", [NPAD, 128], f16)
    W1 = inp("W1", [128, 128], f16)
    W2 = inp("W2", [128, 128], f16)
    b1r = inp("b1r", [1, 128], f16)
    b2c = inp("b2c", [128, 1], f32)
    ones16 = inp("ones16", [1, 128], f16)
    gam = inp("gamma", [128, 1], f32)
    bet = inp("beta", [128, 1], f32)
    idxp = inp("idxp", [128, T * SW], i16)
    iotaT = inp("iotaT", [128, NCH * 128], f16)
    colj = inp("colj", [128, T * NCH], f16)
    normv = inp("normv", [128, T * NCH], f16)
    xown = inp("xown", [T * 128, 128], f16)
    cnts = inp("cnts", [1, T * 2], mybir.dt.int32)

    y = nc.dram_tensor("y", [128, NL], f32, kind="ExternalOutput").ap()

    h1_my = nc.dram_tensor("h1_my", [NL, 128], f16)
    h1_all = nc.dram_tensor("h1_all", [N, 128], f16, addr_space="Shared")
    arin = nc.dram_tensor("arin", [128, 2], f32)
    arout = nc.dram_tensor("arout", [128, 2], f32, addr_space="Shared")

    Relu = mybir.ActivationFunctionType.Relu
    Copy = mybir.ActivationFunctionType.Copy
    Ident = mybir.ActivationFunctionType.Identity
    Square = mybir.ActivationFunctionType.Square
    Sqrt = mybir.ActivationFunctionType.Sqrt
    ADD = mybir.AluOpType.add
    EQ = mybir.AluOpType.is_equal
    MUL = mybir.AluOpType.mult

    qctr = [0]

    def next_q():
        q = qctr[0] & 3
        qctr[0] += 1
        return q

    with tile.TileContext(nc) as tc:
        with tc.tile_pool(name="const", bufs=1) as cp:
            W1_t = cp.tile([128, 128], f16)
            W2_t = cp.tile([128, 128], f16)
            b1_t = cp.tile([1, 128], f16)
            b2_t = cp.tile([128, 1], f32)
            ones16_t = cp.tile([1, 128], f16)
            gam_t = cp.tile([128, 1], f32)
            bet_t = cp.tile([128, 1], f32)
            idxp_t = cp.tile([128, T * SW], i16)
            iotaT_t = cp.tile([128, NCH, 128], f16)
            colj_t = cp.tile([128, T * NCH], f16)
            normv_t = cp.tile([128, T * NCH], f16)
            h2T = cp.tile([128, T, 128], f16)      # resident layer-2 output
            sums = cp.tile([128, T], f32)          # per-tile feature sums
            sumsq = cp.tile([128, T], f32)         # per-tile feature sum-sq
            cnts_t = cp.tile([1, T * 2], mybir.dt.int32)

            for dst, src in [
                (W1_t, W1), (W2_t, W2), (b1_t, b1r), (b2_t, b2c),
                (ones16_t, ones16),
                (gam_t, gam), (bet_t, bet), (idxp_t, idxp),
                (colj_t, colj), (normv_t, normv), (cnts_t, cnts),
            ]:
                nc.sync.dma_start(dst[:], src)
            nc.sync.dma_start(
                iotaT_t[:], iotaT.rearrange("p (c f) -> p c f", c=NCH))

            regA = nc.gpsimd.alloc_register("cntA")
            regB = nc.gpsimd.alloc_register("cntB")

            def aggregate_tile(t, srcA, srcB, selfsrc, selfrows, wp, pp):
                """Gather + one-hot matmuls for target tile t.
                Returns aggT [feat, tgt] f16 SBUF tile."""
                gt = wp.tile([128, NCH, 128], f16, tag="gt")
                nc.gpsimd.reg_load(regA, cnts_t[0:1, 2 * t:2 * t + 1])
                nc.gpsimd.reg_load(regB, cnts_t[0:1, 2 * t + 1:2 * t + 2])
                nc.gpsimd.dma_gather(
                    gt[:, 0:NCHA, :], srcA,
                    idxp_t[:, t * SW: t * SW + NCHA * 8],
                    NCHA * 128, NCHA * 128, 128, single_packet=False,
                    queue_num=next_q())
                nc.gpsimd.dma_gather(
                    gt[:, NCHA:NCHA + NCHB, :], srcB,
                    idxp_t[:, t * SW + NCHA * 8: (t + 1) * SW],
                    NCHB * 128, NCHB * 128, 128, single_packet=False,
                    queue_num=next_q())
                r0 = t * 128
                nrows = min(128, selfrows - r0)
                nc.sync.dma_start(gt[0:nrows, NCH - 1, :],
                                  selfsrc[r0:r0 + nrows, :])
                S = wp.tile([128, NCH, 128], f16, tag="S")
                cjb = colj_t[:, t * NCH:(t + 1) * NCH][:, :, None]\
                    .broadcast_to([128, NCH, 128])
                nvb = normv_t[:, t * NCH:(t + 1) * NCH][:, :, None]\
                    .broadcast_to([128, NCH, 128])
                nc.vector.scalar_tensor_tensor(S[:], iotaT_t[:], 0.0, cjb,
                                               ADD, EQ)
                nc.vector.scalar_tensor_tensor(S[:], S[:], 0.0, nvb,
                                               ADD, MUL)
                ps = pp.tile([128, 128], f32, tag="psagg")
                for ch in range(NCH):
                    nc.tensor.matmul(ps[:], gt[:, ch, :], S[:, ch, :],
                                     start=(ch == 0), stop=(ch == NCH - 1))
                aggT = wp.tile([128, 128], f16, tag="aggT")
                nc.scalar.activation(aggT[:], ps[:], Copy)
                return aggT

            # ================= layer 1 =================
            with (
                tc.tile_pool(name="wp1", bufs=6) as wp1,
                tc.tile_pool(name="pp1", bufs=4, space="PSUM") as pp1,
            ):
                stage = [None]
                for t in range(T):
                    aggT = aggregate_tile(t, x16[0:BUCKET, :],
                                          x16[BUCKET:NPAD, :],
                                          xown, T * 128, wp1, pp1)
                    psh = pp1.tile([128, 128], f32, tag="psh")
                    nc.tensor.matmul(psh[:], aggT[:], W1_t[:],
                                     start=True, stop=False)
                    nc.tensor.matmul(psh[:], ones16_t[:], b1_t[:],
                                     start=False, stop=True)
                    i = t % 8
                    if i == 0:
                        stage[0] = wp1.tile([128, 8, 128], f16, tag="h1st",
                                            name="h1st")
                    nc.scalar.activation(stage[0][:, i, :], psh[:], Relu)
                    if i == 7 or t == T - 1:
                        t0 = t - i
                        r0 = t0 * 128
                        nb = i + 1
                        if t < T - 1:
                            dst = h1_my.ap()[r0:r0 + nb * 128, :].rearrange(
                                "(i p) f -> p i f", p=128)
                            nc.sync.dma_start(dst, stage[0][:, 0:nb, :])
                        else:
                            if nb > 1:
                                dst = h1_my.ap()[r0:r0 + (nb - 1) * 128, :]\
                                    .rearrange("(i p) f -> p i f", p=128)
                                nc.sync.dma_start(dst, stage[0][:, 0:nb - 1, :])
                            r1 = r0 + (nb - 1) * 128
                            nc.sync.dma_start(h1_my.ap()[r1:r1 + LAST, :],
                                              stage[0][0:LAST, nb - 1, :])

            nc.gpsimd.collective_compute(
                "AllGather", mybir.AluOpType.bypass,
                replica_groups=[list(range(C))],
                ins=[h1_my.ap()], outs=[h1_all.ap()])

            # ================= layer 2 =================
            with (
                tc.tile_pool(name="wp2", bufs=6) as wp2,
                tc.tile_pool(name="pp2", bufs=4, space="PSUM") as pp2,
            ):
                for t in range(T):
                    aggT = aggregate_tile(t, h1_all.ap()[0:BUCKET, :],
                                          h1_all.ap()[BUCKET:N, :],
                                          h1_my.ap(), NL, wp2, pp2)
                    psh = pp2.tile([128, 128], f32, tag="psh2")
                    nc.tensor.matmul(psh[:], W2_t[:], aggT[:],
                                     start=True, stop=True)
                    sqd = wp2.tile([128, 128], f16, tag="sqd")
                    if t < T - 1:
                        nc.scalar.activation(h2T[:, t, :], psh[:], Relu,
                                             bias=b2_t[:],
                                             accum_out=sums[:, t:t + 1])
                        nc.scalar.activation(sqd[:], h2T[:, t, :], Square,
                                             accum_out=sumsq[:, t:t + 1])
                    else:
                        nc.scalar.activation(h2T[:, t, :], psh[:], Relu,
                                             bias=b2_t[:])
                        nc.vector.memset(h2T[:, t, LAST:128], 0.0)
                        nc.scalar.activation(sqd[:], h2T[:, t, :], Ident,
                                             accum_out=sums[:, t:t + 1])
                        nc.scalar.activation(sqd[:], h2T[:, t, :], Square,
                                             accum_out=sumsq[:, t:t + 1])

            # ================= batch norm =================
            with (
                tc.tile_pool(name="wp5", bufs=3) as wp5,
            ):
                acc = wp5.tile([128, 2], f32, tag="acc")
                nc.vector.reduce_sum(acc[:, 0:1], sums[:],
                                     axis=mybir.AxisListType.X)
                nc.vector.reduce_sum(acc[:, 1:2], sumsq[:],
                                     axis=mybir.AxisListType.X)
                nc.sync.dma_start(arin.ap(), acc[:])
                nc.gpsimd.collective_compute(
                    "AllReduce", mybir.AluOpType.add,
                    replica_groups=[list(range(C))],
                    ins=[arin.ap()], outs=[arout.ap()])
                ar = wp5.tile([128, 2], f32, tag="ar")
                nc.sync.dma_start(ar[:], arout.ap())

                mean = wp5.tile([128, 1], f32, tag="mean")
                ex2 = wp5.tile([128, 1], f32, tag="ex2")
                var = wp5.tile([128, 1], f32, tag="var")
                std = wp5.tile([128, 1], f32, tag="std")
                inv = wp5.tile([128, 1], f32, tag="inv")
                scl = wp5.tile([128, 1], f32, tag="scl")
                sft = wp5.tile([128, 1], f32, tag="sft")

                nc.vector.tensor_scalar_mul(mean[:], ar[:, 0:1], 1.0 / N)
                nc.vector.tensor_scalar_mul(ex2[:], ar[:, 1:2], 1.0 / N)
                nc.vector.tensor_mul(var[:], mean[:], mean[:])
                nc.vector.tensor_sub(var[:], ex2[:], var[:])
                nc.vector.tensor_scalar_add(var[:], var[:], EPS)
                nc.scalar.activation(std[:], var[:], Sqrt)
                nc.vector.reciprocal(inv[:], std[:])
                nc.vector.tensor_mul(scl[:], gam_t[:], inv[:])
                nc.vector.tensor_mul(sft[:], mean[:], scl[:])
                nc.vector.tensor_sub(sft[:], bet_t[:], sft[:])

                done = 0
                while done < T:
                    nb = min(8, T - done)
                    yst = wp5.tile([128, 8, 128], f32, tag="yst")
                    for i in range(nb):
                        t = done + i
                        nc.scalar.activation(yst[:, i, :], h2T[:, t, :],
                                             Ident, bias=sft[:], scale=scl[:])
                    c0 = done * 128
                    if done + nb < T:
                        dst = y[:, c0:c0 + nb * 128].rearrange(
                            "p (i f) -> p i f", i=nb)
                        nc.sync.dma_start(dst, yst[:, 0:nb, :])
                    else:
                        if nb > 1:
                            dst = y[:, c0:c0 + (nb - 1) * 128].rearrange(
                                "p (i f) -> p i f", i=nb - 1)
                            nc.sync.dma_start(dst, yst[:, 0:nb - 1, :])
                        c1 = c0 + (nb - 1) * 128
                        nc.sync.dma_start(y[:, c1:c1 + LAST],
                                          yst[:, nb - 1, 0:LAST])
                    done += nb

    nc.compile()
    return nc


# --------------------------------------------------------------------------
# entry point
# --------------------------------------------------------------------------

def _run(inputs, trace=False):
    x = np.asarray(inputs["x"], dtype=np.float32)
    edge_index = np.asarray(inputs["edge_index"])
    W1 = np.asarray(inputs["W1"], dtype=np.float32)
    b1 = np.asarray(inputs["b1"], dtype=np.float32)
    W2 = np.asarray(inputs["W2"], dtype=np.float32)
    b2 = np.asarray(inputs["b2"], dtype=np.float32)
    gamma = np.asarray(inputs["gamma"], dtype=np.float32)
    beta = np.asarray(inputs["beta"], dtype=np.float32)

    NCHA, NCHB, idxp, colj, normv, cnts = _prep_edges(edge_index)
    key = (NCHA, NCHB)
    if key not in _BUILD_CACHE:
        _BUILD_CACHE[key] = _build(NCHA, NCHB)
    nc = _BUILD_CACHE[key]

    xp = np.zeros((NPAD, D), np.float16)
    xp[:N] = x.astype(np.float16)
    xown = np.zeros((C, T * 128, D), np.float16)
    xown[:, :NL] = xp[:N].reshape(C, NL, D)

    NCH = NCHA + NCHB + 1
    iotaT = np.ascontiguousarray(
        np.tile(np.arange(128, dtype=np.float16), (128, NCH)))

    common = {
        "x16": xp,
        "W1": W1.astype(np.float16),
        "W2": W2.astype(np.float16),
        "b1r": b1.astype(np.float16)[None, :],
        "b2c": b2.astype(np.float32)[:, None],
        "ones16": np.ones((1, 128), np.float16),
        "gamma": gamma.astype(np.float32).reshape(128, 1),
        "beta": beta.astype(np.float32).reshape(128, 1),
        "iotaT": iotaT,
    }
    in_maps = [
        {**common, "idxp": idxp[c], "colj": colj[c], "normv": normv[c],
         "xown": xown[c], "cnts": cnts[c].reshape(1, T * 2)}
        for c in range(C)
    ]

    res = run_bass_kernel_spmd(nc, in_maps, list(range(C)), trace=trace)
    out = np.concatenate(
        [np.ascontiguousarray(res.results[c]["y"].T) for c in range(C)], axis=0)
    return out, res


def kernel(**inputs):
    out, _ = _run(inputs, trace=False)
    return out



# revision 38
# speedup vs baseline: 1.9332x; 1.1139x over previous
"""GCN block (2x GCNConv + BatchNorm) on 8 Trainium2 NeuronCores — v4.

Design vs v2:
- S-build batched: the one-hot scatter matrices S [slot, tgt] (norm baked in)
  are built with TWO DVE scalar_tensor_tensor ops per target tile (all NCH
  chunks at once, per-chunk colj/norm broadcast along the free dim via
  stride-0 APs) instead of v2's NCH separate tensor_scalar ops. v2's trace
  showed the per-chunk DVE builds at ~100% occupancy (critical path ~1.2 ms
  of 1.49 ms). Streaming host-built S from DRAM (v3) was tried and is WORSE:
  +50 MB HBM traffic trips the power governor (throttle_active 608 us at 50%
  duty) and skews the AllGather across cores.
- Gather slots sorted by source row within each (tile, bucket) for better
  HBM row locality in the per-edge gathers.
- Everything else as v2: per-edge dma_gather from node-major f16 source
  (x16 for layer 1, AllGather'd h1 for layer 2) on 4 SWDGE queues; one-hot
  matmuls accumulate [feat, tgt] in PSUM; one dense matmul per 128-node tile;
  layer 2 feature-major with bias+relu fused in ACT, BN stats via accum_out.
"""

import numpy as np

import concourse.bacc as bacc
import concourse.mybir as mybir
import concourse.tile as tile
from concourse.bass_utils import run_bass_kernel_spmd

N, E, D = 50000, 600000, 128
C = 8                      # cores
NL = N // C                # 6250 nodes per core
T = (NL + 127) // 128      # 49 target tiles per core
LAST = NL - (T - 1) * 128  # 106 valid rows in the last tile
NPAD = ((N + 127) // 128) * 128  # 50048
BUCKET = 32768             # int16-safe source split
EPS = 1e-5

f16 = mybir.dt.float16
f32 = mybir.dt.float32
i16 = mybir.dt.int16

_BUILD_CACHE = {}


# --------------------------------------------------------------------------
# host-side preprocessing (same edge bucketing as v1)
# --------------------------------------------------------------------------

def _prep_edges(edge_index):
    row = np.asarray(edge_index[0], dtype=np.int64)
    col = np.asarray(edge_index[1], dtype=np.int64)
    deg = np.bincount(col, minlength=N).astype(np.float32) + 1.0
    dis = (1.0 / np.sqrt(deg)).astype(np.float32)

    # real edges only; self-loops handled by a dense diagonal chunk
    rows = row
    cols = col
    norm = (dis[row] * dis[col]).astype(np.float32)

    core = cols // NL
    col_loc = cols - core * NL
    t = col_loc >> 7
    b = (rows >= BUCKET).astype(np.int64)
    idxv = (rows - b * BUCKET).astype(np.int16)

    gid = (core * T + t) * 2 + b
    order = np.lexsort((rows, gid))  # by group, then source row
    gid_s = gid[order]
    counts = np.bincount(gid_s, minlength=C * T * 2)
    starts = np.concatenate([[0], np.cumsum(counts)[:-1]])
    rank = (np.arange(len(gid_s)) - starts[gid_s]).astype(np.int64)

    cnt = counts.reshape(C, T, 2)
    NCHA = max(1, int(-(-cnt[:, :, 0].max() // 128)))
    NCHB = max(1, int(-(-cnt[:, :, 1].max() // 128)))
    NCH = NCHA + NCHB + 1      # +1: dense self-loop (diagonal) chunk
    SW = (NCHA + NCHB) * 8

    core_s = core[order]
    t_s = t[order]
    b_s = b[order]
    j_s = (col_loc & 127)[order]
    idx_s = idxv[order]
    norm_s = norm[order]

    swt = np.where(b_s == 1, NCHA * 128, 0) + rank
    p = (swt & 127).astype(np.int64)
    ch = (swt >> 7).astype(np.int64)

    # per-chunk one-hot params: target column and norm per (slot, tile, chunk)
    colj = np.zeros((C, 128, T * NCH), np.float16)
    normv = np.zeros((C, 128, T * NCH), np.float16)
    colj[core_s, p, t_s * NCH + ch] = j_s
    normv[core_s, p, t_s * NCH + ch] = norm_s.astype(np.float16)

    # diagonal self-loop chunk: slot p aggregates into target column p with
    # weight dis^2 of the tile's own node (0 on pad rows of the last tile)
    pp = np.arange(128)
    colj[:, :, NCH - 1::NCH] = pp[None, :, None].astype(np.float16)
    dis2 = np.zeros((C, T * 128), np.float32)
    dis2[:, :NL] = (dis * dis).reshape(C, NL)
    normv[:, :, NCH - 1::NCH] = dis2.reshape(C, T, 128).transpose(
        0, 2, 1).astype(np.float16)

    idxp16 = np.zeros((C, 16, T * SW), np.int16)
    base = np.where(b_s == 1, NCHA * 8, 0)
    idxp16[core_s, rank & 15, t_s * SW + base + (rank >> 4)] = idx_s
    idxp = np.ascontiguousarray(np.tile(idxp16, (1, 8, 1)))

    # exact per-(tile, bucket) gather counts (>=1 so the ucode path is sane)
    cnts = np.maximum(cnt, 1).astype(np.int32)  # [C, T, 2]

    return NCHA, NCHB, idxp, colj, normv, cnts


# --------------------------------------------------------------------------
# device program
# --------------------------------------------------------------------------

def _build(NCHA, NCHB):
    NCH = NCHA + NCHB + 1      # +1: dense self-loop (diagonal) chunk
    SW = (NCHA + NCHB) * 8

    nc = bacc.Bacc("TRN2", target_bir_lowering=False, debug=False,
                   num_devices=C, num_swdge_queues=4,
                   dynamic_dma_scratch_size=49152)

    def inp(name, shape, dt):
        return nc.dram_tensor(name, shape, dt, kind="ExternalInput").ap()

    x16 = inp("x16", [NPAD, 128], f16)
    W1 = inp("W1", [128, 128], f16)
    W2 = inp("W2", [128, 128], f16)
    b1r = inp("b1r", [1, 128], f16)
    b2c = inp("b2c", [128, 1], f32)
    ones16 = inp("ones16", [1, 128], f16)
    gam = inp("gamma", [128, 1], f32)
    bet = inp("beta", [128, 1], f32)
    idxp = inp("idxp", [128, T * SW], i16)
    iotaT = inp("iotaT", [128, NCH * 128], f16)
    colj = inp("colj", [128, T * NCH], f16)
    normv = inp("normv", [128, T * NCH], f16)
    xown = inp("xown", [T * 128, 128], f16)
    cnts = inp("cnts", [1, T * 2], mybir.dt.int32)

    y = nc.dram_tensor("y", [128, NL], f32, kind="ExternalOutput").ap()

    h1_my = nc.dram_tensor("h1_my", [NL, 128], f16)
    h1_all = nc.dram_tensor("h1_all", [N, 128], f16, addr_space="Shared")
    h1_io = nc.dram_tensor("h1_io", [N, 128], f16,
                           kind="ExternalOutput").ap()  # io-region gather src
    arin = nc.dram_tensor("arin", [128, 2], f32)
    arout = nc.dram_tensor("arout", [128, 2], f32, addr_space="Shared")

    Relu = mybir.ActivationFunctionType.Relu
    Copy = mybir.ActivationFunctionType.Copy
    Ident = mybir.ActivationFunctionType.Identity
    Square = mybir.ActivationFunctionType.Square
    Sqrt = mybir.ActivationFunctionType.Sqrt
    ADD = mybir.AluOpType.add
    EQ = mybir.AluOpType.is_equal
    MUL = mybir.AluOpType.mult

    qctr = [0]

    def next_q():
        q = qctr[0] & 3
        qctr[0] += 1
        return q

    with tile.TileContext(nc) as tc:
        with tc.tile_pool(name="const", bufs=1) as cp:
            W1_t = cp.tile([128, 128], f16)
            W2_t = cp.tile([128, 128], f16)
            b1_t = cp.tile([1, 128], f16)
            b2_t = cp.tile([128, 1], f32)
            ones16_t = cp.tile([1, 128], f16)
            gam_t = cp.tile([128, 1], f32)
            bet_t = cp.tile([128, 1], f32)
            idxp_t = cp.tile([128, T * SW], i16)
            iotaT_t = cp.tile([128, NCH, 128], f16)
            colj_t = cp.tile([128, T * NCH], f16)
            normv_t = cp.tile([128, T * NCH], f16)
            h2T = cp.tile([128, T, 128], f16)      # resident layer-2 output
            sums = cp.tile([128, T], f32)          # per-tile feature sums
            sumsq = cp.tile([128, T], f32)         # per-tile feature sum-sq
            cnts_t = cp.tile([1, T * 2], mybir.dt.int32)

            for dst, src in [
                (W1_t, W1), (W2_t, W2), (b1_t, b1r), (b2_t, b2c),
                (ones16_t, ones16),
                (gam_t, gam), (bet_t, bet), (idxp_t, idxp),
                (colj_t, colj), (normv_t, normv), (cnts_t, cnts),
            ]:
                nc.sync.dma_start(dst[:], src)
            nc.sync.dma_start(
                iotaT_t[:], iotaT.rearrange("p (c f) -> p c f", c=NCH))

            regA = nc.gpsimd.alloc_register("cntA")
            regB = nc.gpsimd.alloc_register("cntB")

            def aggregate_tile(t, srcA, srcB, selfsrc, selfrows, wp, pp):
                """Gather + one-hot matmuls for target tile t.
                Returns aggT [feat, tgt] f16 SBUF tile."""
                gt = wp.tile([128, NCH, 128], f16, tag="gt")
                nc.gpsimd.reg_load(regA, cnts_t[0:1, 2 * t:2 * t + 1])
                nc.gpsimd.reg_load(regB, cnts_t[0:1, 2 * t + 1:2 * t + 2])
                nc.gpsimd.dma_gather(
                    gt[:, 0:NCHA, :], srcA,
                    idxp_t[:, t * SW: t * SW + NCHA * 8],
                    NCHA * 128, NCHA * 128, 128, single_packet=False,
                    queue_num=next_q())
                nc.gpsimd.dma_gather(
                    gt[:, NCHA:NCHA + NCHB, :], srcB,
                    idxp_t[:, t * SW + NCHA * 8: (t + 1) * SW],
                    NCHB * 128, NCHB * 128, 128, single_packet=False,
                    queue_num=next_q())
                r0 = t * 128
                nrows = min(128, selfrows - r0)
                nc.sync.dma_start(gt[0:nrows, NCH - 1, :],
                                  selfsrc[r0:r0 + nrows, :])
                S = wp.tile([128, NCH, 128], f16, tag="S")
                cjb = colj_t[:, t * NCH:(t + 1) * NCH][:, :, None]\
                    .broadcast_to([128, NCH, 128])
                nvb = normv_t[:, t * NCH:(t + 1) * NCH][:, :, None]\
                    .broadcast_to([128, NCH, 128])
                nc.vector.scalar_tensor_tensor(S[:], iotaT_t[:], 0.0, cjb,
                                               ADD, EQ)
                nc.vector.scalar_tensor_tensor(S[:], S[:], 0.0, nvb,
                                               ADD, MUL)
                ps = pp.tile([128, 128], f32, tag="psagg")
                for ch in range(NCH):
                    nc.tensor.matmul(ps[:], gt[:, ch, :], S[:, ch, :],
                                     start=(ch == 0), stop=(ch == NCH - 1))
                aggT = wp.tile([128, 128], f16, tag="aggT")
                nc.scalar.activation(aggT[:], ps[:], Copy)
                return aggT

            # ================= layer 1 =================
            with (
                tc.tile_pool(name="wp1", bufs=6) as wp1,
                tc.tile_pool(name="pp1", bufs=4, space="PSUM") as pp1,
            ):
                stage = [None]
                for t in range(T):
                    aggT = aggregate_tile(t, x16[0:BUCKET, :],
                                          x16[BUCKET:NPAD, :],
                                          xown, T * 128, wp1, pp1)
                    psh = pp1.tile([128, 128], f32, tag="psh")
                    nc.tensor.matmul(psh[:], aggT[:], W1_t[:],
                                     start=True, stop=False)
                    nc.tensor.matmul(psh[:], ones16_t[:], b1_t[:],
                                     start=False, stop=True)
                    i = t % 8
                    if i == 0:
                        stage[0] = wp1.tile([128, 8, 128], f16, tag="h1st",
                                            name="h1st")
                    nc.scalar.activation(stage[0][:, i, :], psh[:], Relu)
                    if i == 7 or t == T - 1:
                        t0 = t - i
                        r0 = t0 * 128
                        nb = i + 1
                        if t < T - 1:
                            dst = h1_my.ap()[r0:r0 + nb * 128, :].rearrange(
                                "(i p) f -> p i f", p=128)
                            nc.sync.dma_start(dst, stage[0][:, 0:nb, :])
                        else:
                            if nb > 1:
                                dst = h1_my.ap()[r0:r0 + (nb - 1) * 128, :]\
                                    .rearrange("(i p) f -> p i f", p=128)
                                nc.sync.dma_start(dst, stage[0][:, 0:nb - 1, :])
                            r1 = r0 + (nb - 1) * 128
                            nc.sync.dma_start(h1_my.ap()[r1:r1 + LAST, :],
                                              stage[0][0:LAST, nb - 1, :])

            nc.gpsimd.collective_compute(
                "AllGather", mybir.AluOpType.bypass,
                replica_groups=[list(range(C))],
                ins=[h1_my.ap()], outs=[h1_all.ap()])
            nc.sync.dma_start(
                h1_io.rearrange("(a b) f -> a (b f)", a=1),
                h1_all.ap().rearrange("(a b) f -> a (b f)", a=1))

            # ================= layer 2 =================
            with (
                tc.tile_pool(name="wp2", bufs=6) as wp2,
                tc.tile_pool(name="pp2", bufs=4, space="PSUM") as pp2,
            ):
                for t in range(T):
                    aggT = aggregate_tile(t, h1_io[0:BUCKET, :],
                                          h1_io[BUCKET:N, :],
                                          h1_my.ap(), NL, wp2, pp2)
                    psh = pp2.tile([128, 128], f32, tag="psh2")
                    nc.tensor.matmul(psh[:], W2_t[:], aggT[:],
                                     start=True, stop=True)
                    sqd = wp2.tile([128, 128], f16, tag="sqd")
                    if t < T - 1:
                        nc.scalar.activation(h2T[:, t, :], psh[:], Relu,
                                             bias=b2_t[:],
                                             accum_out=sums[:, t:t + 1])
                        nc.scalar.activation(sqd[:], h2T[:, t, :], Square,
                                             accum_out=sumsq[:, t:t + 1])
                    else:
                        nc.scalar.activation(h2T[:, t, :], psh[:], Relu,
                                             bias=b2_t[:])
                        nc.vector.memset(h2T[:, t, LAST:128], 0.0)
                        nc.scalar.activation(sqd[:], h2T[:, t, :], Ident,
                                             accum_out=sums[:, t:t + 1])
                        nc.scalar.activation(sqd[:], h2T[:, t, :], Square,
                                             accum_out=sumsq[:, t:t + 1])

            # ================= batch norm =================
            with (
                tc.tile_pool(name="wp5", bufs=3) as wp5,
            ):
                acc = wp5.tile([128, 2], f32, tag="acc")
                nc.vector.reduce_sum(acc[:, 0:1], sums[:],
                                     axis=mybir.AxisListType.X)
                nc.vector.reduce_sum(acc[:, 1:2], sumsq[:],
                                     axis=mybir.AxisListType.X)
                nc.sync.dma_start(arin.ap(), acc[:])
                nc.gpsimd.collective_compute(
                    "AllReduce", mybir.AluOpType.add,
                    replica_groups=[list(range(C))],
                    ins=[arin.ap()], outs=[arout.ap()])
                ar = wp5.tile([128, 2], f32, tag="ar")
                nc.sync.dma_start(ar[:], arout.ap())

                mean = wp5.tile([128, 1], f32, tag="mean")
                ex2 = wp5.tile([128, 1], f32, tag="ex2")
                var = wp5.tile([128, 1], f32, tag="var")
                std = wp5.tile([128, 1], f32, tag="std")
                inv = wp5.tile([128, 1], f32, tag="inv")
                scl = wp5.tile([128, 1], f32, tag="scl")
                sft = wp5.tile([128, 1], f32, tag="sft")

                nc.vector.tensor_scalar_mul(mean[:], ar[:, 0:1], 1.0 / N)
                nc.vector.tensor_scalar_mul(ex2[:], ar[:, 1:2], 1.0 / N)
                nc.vector.tensor_mul(var[:], mean[:], mean[:])
                nc.vector.tensor_sub(var[:], ex2[:], var[:])
                nc.vector.tensor_scalar_add(var[:], var[:], EPS)
                nc.scalar.activation(std[:], var[:], Sqrt)
                nc.vector.reciprocal(inv[:], std[:])
                nc.vector.tensor_mul(scl[:], gam_t[:], inv[:])
                nc.vector.tensor_mul(sft[:], mean[:], scl[:])
                nc.vector.tensor_sub(sft[:], bet_t[:], sft[:])

                done = 0
                while done < T:
                    nb = min(8, T - done)
                    yst = wp5.tile([128, 8, 128], f32, tag="yst")
                    for i in range(nb):
                        t = done + i
                        nc.scalar.activation(yst[:, i, :], h2T[:, t, :],
                                             Ident, bias=sft[:], scale=scl[:])
                    c0 = done * 128
                    if done + nb < T:
                        dst = y[:, c0:c0 + nb * 128].rearrange(
                            "p (i f) -> p i f", i=nb)
                        nc.sync.dma_start(dst, yst[:, 0:nb, :])
                    else:
                        if nb > 1:
                            dst = y[:, c0:c0 + (nb - 1) * 128].rearrange(
                                "p (i f) -> p i f", i=nb - 1)
                            nc.sync.dma_start(dst, yst[:, 0:nb - 1, :])
                        c1 = c0 + (nb - 1) * 128
                        nc.sync.dma_start(y[:, c1:c1 + LAST],
                                          yst[:, nb - 1, 0:LAST])
                    done += nb

    nc.compile()
    return nc


# --------------------------------------------------------------------------
# entry point
# --------------------------------------------------------------------------

def _run(inputs, trace=False):
    x = np.asarray(inputs["x"], dtype=np.float32)
    edge_index = np.asarray(inputs["edge_index"])
    W1 = np.asarray(inputs["W1"], dtype=np.float32)
    b1 = np.asarray(inputs["b1"], dtype=np.float32)
    W2 = np.asarray(inputs["W2"], dtype=np.float32)
    b2 = np.asarray(inputs["b2"], dtype=np.float32)
    gamma = np.asarray(inputs["gamma"], dtype=np.float32)
    beta = np.asarray(inputs["beta"], dtype=np.float32)

    NCHA, NCHB, idxp, colj, normv, cnts = _prep_edges(edge_index)
    key = (NCHA, NCHB)
    if key not in _BUILD_CACHE:
        _BUILD_CACHE[key] = _build(NCHA, NCHB)
    nc = _BUILD_CACHE[key]

    xp = np.zeros((NPAD, D), np.float16)
    xp[:N] = x.astype(np.float16)
    xown = np.zeros((C, T * 128, D), np.float16)
    xown[:, :NL] = xp[:N].reshape(C, NL, D)

    NCH = NCHA + NCHB + 1
    iotaT = np.ascontiguousarray(
        np.tile(np.arange(128, dtype=np.float16), (128, NCH)))

    common = {
        "x16": xp,
        "W1": W1.astype(np.float16),
        "W2": W2.astype(np.float16),
        "b1r": b1.astype(np.float16)[None, :],
        "b2c": b2.astype(np.float32)[:, None],
        "ones16": np.ones((1, 128), np.float16),
        "gamma": gamma.astype(np.float32).reshape(128, 1),
        "beta": beta.astype(np.float32).reshape(128, 1),
        "iotaT": iotaT,
    }
    in_maps = [
        {**common, "idxp": idxp[c], "colj": colj[c], "normv": normv[c],
         "xown": xown[c], "cnts": cnts[c].reshape(1, T * 2)}
        for c in range(C)
    ]

    res = run_bass_kernel_spmd(nc, in_maps, list(range(C)), trace=trace)
    out = np.concatenate(
        [np.ascontiguousarray(res.results[c]["y"].T) for c in range(C)], axis=0)
    return out, res


def kernel(**inputs):
    out, _ = _run(inputs, trace=False)
    return out



# revision 39
# speedup vs baseline: 1.9834x; 1.0260x over previous
"""GCN block (2x GCNConv + BatchNorm) on 8 Trainium2 NeuronCores — v4.

Design vs v2:
- S-build batched: the one-hot scatter matrices S [slot, tgt] (norm baked in)
  are built with TWO DVE scalar_tensor_tensor ops per target tile (all NCH
  chunks at once, per-chunk colj/norm broadcast along the free dim via
  stride-0 APs) instead of v2's NCH separate tensor_scalar ops. v2's trace
  showed the per-chunk DVE builds at ~100% occupancy (critical path ~1.2 ms
  of 1.49 ms). Streaming host-built S from DRAM (v3) was tried and is WORSE:
  +50 MB HBM traffic trips the power governor (throttle_active 608 us at 50%
  duty) and skews the AllGather across cores.
- Gather slots sorted by source row within each (tile, bucket) for better
  HBM row locality in the per-edge gathers.
- Everything else as v2: per-edge dma_gather from node-major f16 source
  (x16 for layer 1, AllGather'd h1 for layer 2) on 4 SWDGE queues; one-hot
  matmuls accumulate [feat, tgt] in PSUM; one dense matmul per 128-node tile;
  layer 2 feature-major with bias+relu fused in ACT, BN stats via accum_out.
"""

import numpy as np

import concourse.bacc as bacc
import concourse.mybir as mybir
import concourse.tile as tile
from concourse.bass_utils import run_bass_kernel_spmd

N, E, D = 50000, 600000, 128
C = 8                      # cores
NL = N // C                # 6250 nodes per core
T = (NL + 127) // 128      # 49 target tiles per core
LAST = NL - (T - 1) * 128  # 106 valid rows in the last tile
NPAD = ((N + 127) // 128) * 128  # 50048
BUCKET = 32768             # int16-safe source split
EPS = 1e-5

f16 = mybir.dt.float16
f32 = mybir.dt.float32
i16 = mybir.dt.int16

_BUILD_CACHE = {}
_H1ZERO = np.zeros((N, 128), np.float16)


# --------------------------------------------------------------------------
# host-side preprocessing (same edge bucketing as v1)
# --------------------------------------------------------------------------

def _prep_edges(edge_index):
    row = np.asarray(edge_index[0], dtype=np.int64)
    col = np.asarray(edge_index[1], dtype=np.int64)
    deg = np.bincount(col, minlength=N).astype(np.float32) + 1.0
    dis = (1.0 / np.sqrt(deg)).astype(np.float32)

    # real edges only; self-loops handled by a dense diagonal chunk
    rows = row
    cols = col
    norm = (dis[row] * dis[col]).astype(np.float32)

    core = cols // NL
    col_loc = cols - core * NL
    t = col_loc >> 7
    b = (rows >= BUCKET).astype(np.int64)
    idxv = (rows - b * BUCKET).astype(np.int16)

    gid = (core * T + t) * 2 + b
    order = np.lexsort((rows, gid))  # by group, then source row
    gid_s = gid[order]
    counts = np.bincount(gid_s, minlength=C * T * 2)
    starts = np.concatenate([[0], np.cumsum(counts)[:-1]])
    rank = (np.arange(len(gid_s)) - starts[gid_s]).astype(np.int64)

    cnt = counts.reshape(C, T, 2)
    NCHA = max(1, int(-(-cnt[:, :, 0].max() // 128)))
    NCHB = max(1, int(-(-cnt[:, :, 1].max() // 128)))
    NCH = NCHA + NCHB + 1      # +1: dense self-loop (diagonal) chunk
    SW = (NCHA + NCHB) * 8

    core_s = core[order]
    t_s = t[order]
    b_s = b[order]
    j_s = (col_loc & 127)[order]
    idx_s = idxv[order]
    norm_s = norm[order]

    swt = np.where(b_s == 1, NCHA * 128, 0) + rank
    p = (swt & 127).astype(np.int64)
    ch = (swt >> 7).astype(np.int64)

    # per-chunk one-hot params: target column and norm per (slot, tile, chunk)
    colj = np.zeros((C, 128, T * NCH), np.float16)
    normv = np.zeros((C, 128, T * NCH), np.float16)
    colj[core_s, p, t_s * NCH + ch] = j_s
    normv[core_s, p, t_s * NCH + ch] = norm_s.astype(np.float16)

    # diagonal self-loop chunk: slot p aggregates into target column p with
    # weight dis^2 of the tile's own node (0 on pad rows of the last tile)
    pp = np.arange(128)
    colj[:, :, NCH - 1::NCH] = pp[None, :, None].astype(np.float16)
    dis2 = np.zeros((C, T * 128), np.float32)
    dis2[:, :NL] = (dis * dis).reshape(C, NL)
    normv[:, :, NCH - 1::NCH] = dis2.reshape(C, T, 128).transpose(
        0, 2, 1).astype(np.float16)

    idxp16 = np.zeros((C, 16, T * SW), np.int16)
    base = np.where(b_s == 1, NCHA * 8, 0)
    idxp16[core_s, rank & 15, t_s * SW + base + (rank >> 4)] = idx_s
    idxp = np.ascontiguousarray(np.tile(idxp16, (1, 8, 1)))

    # exact per-(tile, bucket) gather counts (>=1 so the ucode path is sane)
    cnts = np.maximum(cnt, 1).astype(np.int32)  # [C, T, 2]

    return NCHA, NCHB, idxp, colj, normv, cnts


# --------------------------------------------------------------------------
# device program
# --------------------------------------------------------------------------

def _build(NCHA, NCHB):
    NCH = NCHA + NCHB + 1      # +1: dense self-loop (diagonal) chunk
    SW = (NCHA + NCHB) * 8

    nc = bacc.Bacc("TRN2", target_bir_lowering=False, debug=False,
                   num_devices=C, num_swdge_queues=4,
                   dynamic_dma_scratch_size=49152)

    def inp(name, shape, dt):
        return nc.dram_tensor(name, shape, dt, kind="ExternalInput").ap()

    x16 = inp("x16", [NPAD, 128], f16)
    W1 = inp("W1", [128, 128], f16)
    W2 = inp("W2", [128, 128], f16)
    b1r = inp("b1r", [1, 128], f16)
    b2c = inp("b2c", [128, 1], f32)
    ones16 = inp("ones16", [1, 128], f16)
    gam = inp("gamma", [128, 1], f32)
    bet = inp("beta", [128, 1], f32)
    idxp = inp("idxp", [128, T * SW], i16)
    iotaT = inp("iotaT", [128, NCH * 128], f16)
    colj = inp("colj", [128, T * NCH], f16)
    normv = inp("normv", [128, T * NCH], f16)
    xown = inp("xown", [T * 128, 128], f16)
    cnts = inp("cnts", [1, T * 2], mybir.dt.int32)

    y = nc.dram_tensor("y", [128, NL], f32, kind="ExternalOutput").ap()

    h1_my = nc.dram_tensor("h1_my", [NL, 128], f16)
    h1_all = nc.dram_tensor("h1_all", [N, 128], f16, addr_space="Shared")
    h1_io = inp("h1_io", [N, 128], f16)  # io-region gather src (device-written)
    arin = nc.dram_tensor("arin", [128, 2], f32)
    arout = nc.dram_tensor("arout", [128, 2], f32, addr_space="Shared")

    Relu = mybir.ActivationFunctionType.Relu
    Copy = mybir.ActivationFunctionType.Copy
    Ident = mybir.ActivationFunctionType.Identity
    Square = mybir.ActivationFunctionType.Square
    Sqrt = mybir.ActivationFunctionType.Sqrt
    ADD = mybir.AluOpType.add
    EQ = mybir.AluOpType.is_equal
    MUL = mybir.AluOpType.mult

    qctr = [0]

    def next_q():
        q = qctr[0] & 3
        qctr[0] += 1
        return q

    with tile.TileContext(nc) as tc:
        with tc.tile_pool(name="const", bufs=1) as cp:
            W1_t = cp.tile([128, 128], f16)
            W2_t = cp.tile([128, 128], f16)
            b1_t = cp.tile([1, 128], f16)
            b2_t = cp.tile([128, 1], f32)
            ones16_t = cp.tile([1, 128], f16)
            gam_t = cp.tile([128, 1], f32)
            bet_t = cp.tile([128, 1], f32)
            idxp_t = cp.tile([128, T * SW], i16)
            iotaT_t = cp.tile([128, NCH, 128], f16)
            colj_t = cp.tile([128, T * NCH], f16)
            normv_t = cp.tile([128, T * NCH], f16)
            h2T = cp.tile([128, T, 128], f16)      # resident layer-2 output
            sums = cp.tile([128, T], f32)          # per-tile feature sums
            sumsq = cp.tile([128, T], f32)         # per-tile feature sum-sq
            cnts_t = cp.tile([1, T * 2], mybir.dt.int32)

            for dst, src in [
                (W1_t, W1), (W2_t, W2), (b1_t, b1r), (b2_t, b2c),
                (ones16_t, ones16),
                (gam_t, gam), (bet_t, bet), (idxp_t, idxp),
                (colj_t, colj), (normv_t, normv), (cnts_t, cnts),
            ]:
                nc.sync.dma_start(dst[:], src)
            nc.sync.dma_start(
                iotaT_t[:], iotaT.rearrange("p (c f) -> p c f", c=NCH))

            regA = nc.gpsimd.alloc_register("cntA")
            regB = nc.gpsimd.alloc_register("cntB")

            def aggregate_tile(t, srcA, srcB, selfsrc, selfrows, wp, pp):
                """Gather + one-hot matmuls for target tile t.
                Returns aggT [feat, tgt] f16 SBUF tile."""
                gt = wp.tile([128, NCH, 128], f16, tag="gt")
                nc.gpsimd.reg_load(regA, cnts_t[0:1, 2 * t:2 * t + 1])
                nc.gpsimd.reg_load(regB, cnts_t[0:1, 2 * t + 1:2 * t + 2])
                nc.gpsimd.dma_gather(
                    gt[:, 0:NCHA, :], srcA,
                    idxp_t[:, t * SW: t * SW + NCHA * 8],
                    NCHA * 128, NCHA * 128, 128, single_packet=False,
                    queue_num=next_q())
                nc.gpsimd.dma_gather(
                    gt[:, NCHA:NCHA + NCHB, :], srcB,
                    idxp_t[:, t * SW + NCHA * 8: (t + 1) * SW],
                    NCHB * 128, NCHB * 128, 128, single_packet=False,
                    queue_num=next_q())
                r0 = t * 128
                nrows = min(128, selfrows - r0)
                nc.sync.dma_start(gt[0:nrows, NCH - 1, :],
                                  selfsrc[r0:r0 + nrows, :])
                S = wp.tile([128, NCH, 128], f16, tag="S")
                cjb = colj_t[:, t * NCH:(t + 1) * NCH][:, :, None]\
                    .broadcast_to([128, NCH, 128])
                nvb = normv_t[:, t * NCH:(t + 1) * NCH][:, :, None]\
                    .broadcast_to([128, NCH, 128])
                nc.vector.scalar_tensor_tensor(S[:], iotaT_t[:], 0.0, cjb,
                                               ADD, EQ)
                nc.vector.scalar_tensor_tensor(S[:], S[:], 0.0, nvb,
                                               ADD, MUL)
                ps = pp.tile([128, 128], f32, tag="psagg")
                for ch in range(NCH):
                    nc.tensor.matmul(ps[:], gt[:, ch, :], S[:, ch, :],
                                     start=(ch == 0), stop=(ch == NCH - 1))
                aggT = wp.tile([128, 128], f16, tag="aggT")
                nc.scalar.activation(aggT[:], ps[:], Copy)
                return aggT

            # ================= layer 1 =================
            with (
                tc.tile_pool(name="wp1", bufs=6) as wp1,
                tc.tile_pool(name="pp1", bufs=4, space="PSUM") as pp1,
            ):
                stage = [None]
                for t in range(T):
                    aggT = aggregate_tile(t, x16[0:BUCKET, :],
                                          x16[BUCKET:NPAD, :],
                                          xown, T * 128, wp1, pp1)
                    psh = pp1.tile([128, 128], f32, tag="psh")
                    nc.tensor.matmul(psh[:], aggT[:], W1_t[:],
                                     start=True, stop=False)
                    nc.tensor.matmul(psh[:], ones16_t[:], b1_t[:],
                                     start=False, stop=True)
                    i = t % 8
                    if i == 0:
                        stage[0] = wp1.tile([128, 8, 128], f16, tag="h1st",
                                            name="h1st")
                    nc.scalar.activation(stage[0][:, i, :], psh[:], Relu)
                    if i == 7 or t == T - 1:
                        t0 = t - i
                        r0 = t0 * 128
                        nb = i + 1
                        if t < T - 1:
                            dst = h1_my.ap()[r0:r0 + nb * 128, :].rearrange(
                                "(i p) f -> p i f", p=128)
                            nc.sync.dma_start(dst, stage[0][:, 0:nb, :])
                        else:
                            if nb > 1:
                                dst = h1_my.ap()[r0:r0 + (nb - 1) * 128, :]\
                                    .rearrange("(i p) f -> p i f", p=128)
                                nc.sync.dma_start(dst, stage[0][:, 0:nb - 1, :])
                            r1 = r0 + (nb - 1) * 128
                            nc.sync.dma_start(h1_my.ap()[r1:r1 + LAST, :],
                                              stage[0][0:LAST, nb - 1, :])

            nc.gpsimd.collective_compute(
                "AllGather", mybir.AluOpType.bypass,
                replica_groups=[list(range(C))],
                ins=[h1_my.ap()], outs=[h1_all.ap()])
            nc.sync.dma_start(
                h1_io.rearrange("(a b) f -> a (b f)", a=1),
                h1_all.ap().rearrange("(a b) f -> a (b f)", a=1))

            # ================= layer 2 =================
            with (
                tc.tile_pool(name="wp2", bufs=6) as wp2,
                tc.tile_pool(name="pp2", bufs=4, space="PSUM") as pp2,
            ):
                for t in range(T):
                    aggT = aggregate_tile(t, h1_io[0:BUCKET, :],
                                          h1_io[BUCKET:N, :],
                                          h1_my.ap(), NL, wp2, pp2)
                    psh = pp2.tile([128, 128], f32, tag="psh2")
                    nc.tensor.matmul(psh[:], W2_t[:], aggT[:],
                                     start=True, stop=True)
                    sqd = wp2.tile([128, 128], f16, tag="sqd")
                    if t < T - 1:
                        nc.scalar.activation(h2T[:, t, :], psh[:], Relu,
                                             bias=b2_t[:],
                                             accum_out=sums[:, t:t + 1])
                        nc.scalar.activation(sqd[:], h2T[:, t, :], Square,
                                             accum_out=sumsq[:, t:t + 1])
                    else:
                        nc.scalar.activation(h2T[:, t, :], psh[:], Relu,
                                             bias=b2_t[:])
                        nc.vector.memset(h2T[:, t, LAST:128], 0.0)
                        nc.scalar.activation(sqd[:], h2T[:, t, :], Ident,
                                             accum_out=sums[:, t:t + 1])
                        nc.scalar.activation(sqd[:], h2T[:, t, :], Square,
                                             accum_out=sumsq[:, t:t + 1])

            # ================= batch norm =================
            with (
                tc.tile_pool(name="wp5", bufs=3) as wp5,
            ):
                acc = wp5.tile([128, 2], f32, tag="acc")
                nc.vector.reduce_sum(acc[:, 0:1], sums[:],
                                     axis=mybir.AxisListType.X)
                nc.vector.reduce_sum(acc[:, 1:2], sumsq[:],
                                     axis=mybir.AxisListType.X)
                nc.sync.dma_start(arin.ap(), acc[:])
                nc.gpsimd.collective_compute(
                    "AllReduce", mybir.AluOpType.add,
                    replica_groups=[list(range(C))],
                    ins=[arin.ap()], outs=[arout.ap()])
                ar = wp5.tile([128, 2], f32, tag="ar")
                nc.sync.dma_start(ar[:], arout.ap())

                mean = wp5.tile([128, 1], f32, tag="mean")
                ex2 = wp5.tile([128, 1], f32, tag="ex2")
                var = wp5.tile([128, 1], f32, tag="var")
                std = wp5.tile([128, 1], f32, tag="std")
                inv = wp5.tile([128, 1], f32, tag="inv")
                scl = wp5.tile([128, 1], f32, tag="scl")
                sft = wp5.tile([128, 1], f32, tag="sft")

                nc.vector.tensor_scalar_mul(mean[:], ar[:, 0:1], 1.0 / N)
                nc.vector.tensor_scalar_mul(ex2[:], ar[:, 1:2], 1.0 / N)
                nc.vector.tensor_mul(var[:], mean[:], mean[:])
                nc.vector.tensor_sub(var[:], ex2[:], var[:])
                nc.vector.tensor_scalar_add(var[:], var[:], EPS)
                nc.scalar.activation(std[:], var[:], Sqrt)
                nc.vector.reciprocal(inv[:], std[:])
                nc.vector.tensor_mul(scl[:], gam_t[:], inv[:])
                nc.vector.tensor_mul(sft[:], mean[:], scl[:])
                nc.vector.tensor_sub(sft[:], bet_t[:], sft[:])

                done = 0
                while done < T:
                    nb = min(8, T - done)
                    yst = wp5.tile([128, 8, 128], f32, tag="yst")
                    for i in range(nb):
                        t = done + i
                        nc.scalar.activation(yst[:, i, :], h2T[:, t, :],
                                             Ident, bias=sft[:], scale=scl[:])
                    c0 = done * 128
                    if done + nb < T:
                        dst = y[:, c0:c0 + nb * 128].rearrange(
                            "p (i f) -> p i f", i=nb)
                        nc.sync.dma_start(dst, yst[:, 0:nb, :])
                    else:
                        if nb > 1:
                            dst = y[:, c0:c0 + (nb - 1) * 128].rearrange(
                                "p (i f) -> p i f", i=nb - 1)
                            nc.sync.dma_start(dst, yst[:, 0:nb - 1, :])
                        c1 = c0 + (nb - 1) * 128
                        nc.sync.dma_start(y[:, c1:c1 + LAST],
                                          yst[:, nb - 1, 0:LAST])
                    done += nb

    nc.compile()
    return nc


# --------------------------------------------------------------------------
# entry point
# --------------------------------------------------------------------------

def _run(inputs, trace=False):
    x = np.asarray(inputs["x"], dtype=np.float32)
    edge_index = np.asarray(inputs["edge_index"])
    W1 = np.asarray(inputs["W1"], dtype=np.float32)
    b1 = np.asarray(inputs["b1"], dtype=np.float32)
    W2 = np.asarray(inputs["W2"], dtype=np.float32)
    b2 = np.asarray(inputs["b2"], dtype=np.float32)
    gamma = np.asarray(inputs["gamma"], dtype=np.float32)
    beta = np.asarray(inputs["beta"], dtype=np.float32)

    NCHA, NCHB, idxp, colj, normv, cnts = _prep_edges(edge_index)
    key = (NCHA, NCHB)
    if key not in _BUILD_CACHE:
        _BUILD_CACHE[key] = _build(NCHA, NCHB)
    nc = _BUILD_CACHE[key]

    xp = np.zeros((NPAD, D), np.float16)
    xp[:N] = x.astype(np.float16)
    xown = np.zeros((C, T * 128, D), np.float16)
    xown[:, :NL] = xp[:N].reshape(C, NL, D)

    NCH = NCHA + NCHB + 1
    iotaT = np.ascontiguousarray(
        np.tile(np.arange(128, dtype=np.float16), (128, NCH)))

    common = {
        "x16": xp,
        "W1": W1.astype(np.float16),
        "W2": W2.astype(np.float16),
        "b1r": b1.astype(np.float16)[None, :],
        "b2c": b2.astype(np.float32)[:, None],
        "ones16": np.ones((1, 128), np.float16),
        "gamma": gamma.astype(np.float32).reshape(128, 1),
        "beta": beta.astype(np.float32).reshape(128, 1),
        "iotaT": iotaT,
    }
    in_maps = [
        {**common, "idxp": idxp[c], "colj": colj[c], "normv": normv[c],
         "xown": xown[c], "cnts": cnts[c].reshape(1, T * 2),
         "h1_io": _H1ZERO}
        for c in range(C)
    ]

    res = run_bass_kernel_spmd(nc, in_maps, list(range(C)), trace=trace)
    out = np.concatenate(
        [np.ascontiguousarray(res.results[c]["y"].T) for c in range(C)], axis=0)
    return out, res


def kernel(**inputs):
    out, _ = _run(inputs, trace=False)
    return out

